# revision 1
# baseline (speedup 1.0000x reference)
"""DeepSeek block (MLA attention + shared MLP + 7-expert top-2 MoE) on 8 TRN2
NeuronCores.

Sharding: core c handles batch b=c//4, query block j=c%4 (512 tokens) for
attention/MoE; K/V for the full 2048-token batch slab are computed redundantly
on each of the 4 cores of a batch group (uniform SPMD program, no collectives).
Causality is enforced by 0/1 value masks supplied per core.

Layouts: activations live as [128 partitions = H%128, H//128 chunks, tokens]
("T-layout") so every matmul contraction is on partitions; all weights are
pre-transposed on the host. The MoE down-projection emits [tokens, H] so the
per-token top-2 combine weight is a native per-partition scalar.
"""

import functools

import numpy as np
import ml_dtypes

import concourse.bass as bass
import concourse.tile as tile
from concourse import mybir
from concourse.bass import ds, ts
from concourse.bass_utils import run_bass_kernel_spmd
from concourse.masks import make_identity

f32 = mybir.dt.float32
bf16 = mybir.dt.bfloat16
AF = mybir.ActivationFunctionType
OP = mybir.AluOpType

P = 128
B, T, H, L, F, E = 2, 2048, 1024, 256, 2048, 7
HC, LC, FC = H // P, L // P, F // P  # 8, 2, 16
TT = 512          # own tokens per core
TB = 2048         # batch slab tokens
NTB = TB // 512   # 4 batch token tiles
KC = TB // P      # 16 key chunks
EPS = 1e-5
SCALE = 1.0 / 32.0  # 1/sqrt(H)
N_CORES = 8


def _split_multiwaits(nc, max_waits=1):
    """walrus here supports one sync-wait per instruction; hoist extras onto
    preceding NoOps on the same engine."""
    ctr = 0
    for f in nc.m.functions:
        for bb in f.blocks:
            out = []
            dirty = False
            for inst in bb.instructions:
                si = inst.sync_info
                if si is not None and len(si.on_wait) > max_waits:
                    waits = list(si.on_wait)
                    for w in waits[:-max_waits]:
                        ctr += 1
                        nop = mybir.InstNoOp(name=f"waitnop-{ctr}", ins=[], outs=[])
                        nop.engine = inst.engine
                        nop.sync_info = mybir.SyncInfo(on_wait=[w], on_update=[])
                        out.append(nop)
                    inst.sync_info = mybir.SyncInfo(
                        on_wait=waits[-max_waits:], on_update=list(si.on_update)
                    )
                    dirty = True
                out.append(inst)
            if dirty:
                bb.instructions = out
    return ctr


def _bn_mean_rstd(nc, pool, src_ap, epsb):
    """src_ap [128, 1024] f32 -> mr [128, 2] (mean, rstd) via bn_stats."""
    stats = pool.tile([P, 2, 6], f32, tag="bn_stats")
    nc.vector.bn_stats(out=stats[:, 0], in_=src_ap[:, 0:512])
    nc.vector.bn_stats(out=stats[:, 1], in_=src_ap[:, 512:1024])
    mv = pool.tile([P, 2], f32, tag="bn_mv")
    nc.vector.bn_aggr(out=mv, in_=stats)
    mr = pool.tile([P, 2], f32, tag="bn_mr")
    nc.vector.tensor_copy(out=mr[:, 0:1], in_=mv[:, 0:1])
    nc.scalar.activation(out=mr[:, 1:2], in_=mv[:, 1:2], func=AF.Sqrt, bias=epsb)
    nc.vector.reciprocal(out=mr[:, 1:2], in_=mr[:, 1:2])
    return mr


def build_nc(repeat=1):
    nc = bass.Bass()

    def din(name, shape, dt=bf16):
        return nc.declare_dram_parameter(name, list(shape), dt, isOutput=False)

    xbT = din("xbT", [H, TB], f32)
    xoT = din("xoT", [H, TT], f32)
    xon = din("xon", [TT, H], f32)
    cosb = din("cosb", [H, TB])
    sinb = din("sinb", [H, TB])
    cosq = din("cosq", [H, TT])
    sinq = din("sinq", [H, TT])
    msk = din("msk", [KC, P, TT])
    wkvT = din("wkvT", [H, L])
    wqT = din("wqT", [H, L])
    wvT = din("wvT", [L, H])
    wrqT = din("wrqT", [L, H])
    wrkT = din("wrkT", [H, H])
    woT = din("woT", [H, H])
    wrtT = din("wrtT", [P, HC, E], f32)
    rbias = din("rbias", [1, E], f32)
    wsgT = din("wsgT", [H, F])
    wsuT = din("wsuT", [H, F])
    wsdT = din("wsdT", [F, H])
    iob = din("iob", [P, 256], f32)
    triS = din("triS", [P, P])
    ones2d = din("ones2d", [P, P])
    wegT = din("wegT", [E, H, F])
    weuT = din("weuT", [E, H, F])
    wedT = din("wedT", [E, F, H])
    out = nc.declare_dram_parameter("out", [TT, H], f32, isOutput=True)

    r128 = lambda ap: ap.rearrange("(c p) x -> p c x", p=P)

    with tile.TileContext(nc) as tc:
      for rep in range(repeat):
          cst = tc.alloc_tile_pool(name=f"cst{rep}", bufs=1)
          pp = tc.alloc_tile_pool(name=f"pp{rep}", bufs=1)       # persist: qrope, yn, xpn
          psg = tc.alloc_tile_pool(name=f"psg{rep}", bufs=4, space="PSUM")

          ones128b = cst.tile([P, 1], bf16)
          nc.vector.memset(ones128b, 1.0)
          ones1b = cst.tile([1, P], bf16)
          nc.vector.memset(ones1b, 1.0)
          ones1f = cst.tile([1, P], f32)
          nc.vector.memset(ones1f, 1.0)
          epsb1 = cst.tile([1, 1], f32)
          nc.vector.memset(epsb1, EPS)
          epsb128 = cst.tile([P, 1], f32)
          nc.vector.memset(epsb128, EPS)
          ident = cst.tile([P, P], f32)
          make_identity(nc, ident)
          wrt_sb = cst.tile([P, HC, E], f32)
          nc.sync.dma_start(out=wrt_sb, in_=wrtT[:, :, :])
          rbias_sb = cst.tile([1, E], f32)
          nc.sync.dma_start(out=rbias_sb, in_=rbias[:, :])
          iob_sb = cst.tile([P, 256], f32)
          nc.sync.dma_start(out=iob_sb, in_=iob[:, :])
          triS_sb = cst.tile([P, P], bf16)
          nc.sync.dma_start(out=triS_sb, in_=triS[:, :])
          ones2d_sb = cst.tile([P, P], bf16)
          nc.sync.dma_start(out=ones2d_sb, in_=ones2d[:, :])

          qrope = pp.tile([P, HC, TT], bf16)
          yn = pp.tile([P, HC, TT], bf16)
          xpn = pp.tile([P, TT // P, H], f32)

          bv = tc.alloc_tile_pool(name=f"bv{rep}", bufs=1)
          v_sb = bv.tile([P, KC, H], bf16)
          krope = bv.tile([P, HC, TB], bf16)

          bw = tc.alloc_tile_pool(name=f"bw{rep}", bufs=1)
          wkv_sb = bw.tile([P, HC, L], bf16)
          nc.sync.dma_start(out=wkv_sb, in_=r128(wkvT))
          wq_sb = bw.tile([P, HC, L], bf16)
          nc.sync.dma_start(out=wq_sb, in_=r128(wqT))
          wv_sb = bw.tile([P, LC, H], bf16)
          nc.sync.dma_start(out=wv_sb, in_=r128(wvT))
          wrq_sb = bw.tile([P, LC, H], bf16)
          nc.sync.dma_start(out=wrq_sb, in_=r128(wrqT))
          wrk_sb = bw.tile([P, HC, H], bf16)
          nc.sync.dma_start(out=wrk_sb, in_=r128(wrkT))

          # ================== OWN pipeline: h_own -> q_lat -> qrope =========
          with tc.tile_pool(name=f"own{rep}", bufs=1) as own, \
               tc.tile_pool(name=f"ownt{rep}", bufs=2) as ownt:
              muT = own.tile([1, TT], f32)
              rsT = own.tile([1, TT], f32)
              for tm in range(TT // P):
                  xon_t = ownt.tile([P, H], f32, tag="xon_t")
                  nc.sync.dma_start(out=xon_t, in_=r128(xon)[:, tm])
                  mr = _bn_mean_rstd(nc, ownt, xon_t, epsb128)
                  ptm = psg.tile([1, P], f32, tag="pb1")
                  nc.tensor.transpose(ptm, mr[:, 0:1], ident)
                  nc.vector.tensor_copy(out=muT[:, ts(tm, P)], in_=ptm[0:1, :])
                  ptr = psg.tile([1, P], f32, tag="pb1")
                  nc.tensor.transpose(ptr, mr[:, 1:2], ident)
                  nc.vector.tensor_copy(out=rsT[:, ts(tm, P)], in_=ptr[0:1, :])
              pmu = psg.tile([P, TT], f32, tag="pb1")
              nc.tensor.matmul(pmu, ones1f, muT, start=True, stop=True)
              muB = own.tile([P, TT], f32)
              nc.vector.tensor_copy(out=muB, in_=pmu)
              prs = psg.tile([P, TT], f32, tag="pb1")
              nc.tensor.matmul(prs, ones1f, rsT, start=True, stop=True)
              rsB = own.tile([P, TT], f32)
              nc.vector.tensor_copy(out=rsB, in_=prs)

              h_own = own.tile([P, HC, TT], bf16)
              for hc in range(HC):
                  xoT_t = ownt.tile([P, TT], f32, tag="xoT_t")
                  nc.sync.dma_start(out=xoT_t, in_=r128(xoT)[:, hc])
                  tmp = ownt.tile([P, TT], f32, tag="ot_f32")
                  nc.vector.tensor_tensor(out=tmp, in0=xoT_t, in1=muB,
                                          op=OP.subtract)
                  nc.vector.tensor_tensor(out=h_own[:, hc], in0=tmp, in1=rsB,
                                          op=OP.mult)
              qlat = own.tile([P, LC, TT], bf16)
              for lc in range(LC):
                  pq = psg.tile([P, TT], f32, tag="pb1")
                  for hc in range(HC):
                      nc.tensor.matmul(pq, wq_sb[:, hc, ts(lc, P)], h_own[:, hc],
                                       start=(hc == 0), stop=(hc == HC - 1))
                  nc.scalar.activation(out=qlat[:, lc], in_=pq, func=AF.Copy)
              qrf = own.tile([P, HC, TT], bf16)
              for hm in range(HC):
                  pq = psg.tile([P, TT], f32, tag="pb1")
                  for lc in range(LC):
                      nc.tensor.matmul(pq, wrq_sb[:, lc, ts(hm, P)], qlat[:, lc],
                                       start=(lc == 0), stop=(lc == LC - 1))
                  nc.scalar.activation(out=qrf[:, hm], in_=pq, func=AF.Copy)
              cosq_sb = own.tile([P, HC, TT], bf16)
              nc.sync.dma_start(out=cosq_sb, in_=r128(cosq))
              sinq_sb = own.tile([P, HC, TT], bf16)
              nc.sync.dma_start(out=sinq_sb, in_=r128(sinq))
              for c in range(HC):
                  t1 = ownt.tile([P, TT], bf16, tag="ot_b1")
                  t2 = ownt.tile([P, TT], bf16, tag="ot_b2")
                  nc.vector.tensor_tensor(out=t1, in0=qrf[:, c], in1=cosq_sb[:, c],
                                          op=OP.mult)
                  nc.vector.tensor_tensor(out=t2, in0=qrf[:, (c + 4) % 8],
                                          in1=sinq_sb[:, c], op=OP.mult)
                  nc.vector.tensor_tensor(out=qrope[:, c], in0=t1, in1=t2,
                                          op=OP.subtract if c < 4 else OP.add)

          # ================== BATCH loop: h, kv_lat, v, krope ===============
          with tc.tile_pool(name=f"bst{rep}", bufs=1, space="PSUM") as bst, \
               tc.tile_pool(name=f"bt{rep}", bufs=1) as bt, \
               tc.tile_pool(name=f"btt{rep}", bufs=2) as btt:
              for t in range(NTB):
                  tsl = ts(t, 512)
                  xbb = bt.tile([P, HC, 512], bf16, tag="xbb", bufs=2)
                  psA = bst.tile([1, 512], f32, tag="psA")
                  psB = bst.tile([1, 512], f32, tag="psB")
                  for hc in range(HC):
                      xbh = btt.tile([P, 512], f32, tag="xbh", bufs=3)
                      nc.sync.dma_start(out=xbh, in_=r128(xbT)[:, hc, tsl])
                      nc.scalar.activation(out=xbb[:, hc], in_=xbh, func=AF.Copy)
                      xsq = btt.tile([P, 512], bf16, tag="xsq", bufs=1)
                      nc.vector.tensor_tensor(out=xsq, in0=xbb[:, hc],
                                              in1=xbb[:, hc], op=OP.mult)
                      nc.tensor.matmul(psA, ones128b, xbb[:, hc],
                                       start=(hc == 0), stop=(hc == HC - 1))
                      nc.tensor.matmul(psB, ones128b, xsq,
                                       start=(hc == 0), stop=(hc == HC - 1))
                  mu = btt.tile([1, 512], f32, tag="mu", bufs=1)
                  nc.vector.tensor_scalar_mul(out=mu, in0=psA, scalar1=1.0 / H)
                  mu2 = btt.tile([1, 512], f32, tag="mu2", bufs=1)
                  nc.vector.tensor_tensor(out=mu2, in0=mu, in1=mu, op=OP.mult)
                  var = btt.tile([1, 512], f32, tag="var", bufs=1)
                  nc.vector.scalar_tensor_tensor(out=var, in0=psB, scalar=1.0 / H,
                                                 in1=mu2, op0=OP.mult, op1=OP.subtract)
                  nc.scalar.activation(out=var, in_=var, func=AF.Sqrt, bias=epsb1)
                  rs = var
                  nc.vector.reciprocal(out=rs, in_=rs)
                  mub = btt.tile([1, 512], bf16, tag="mub", bufs=1)
                  nc.vector.tensor_copy(out=mub, in_=mu)
                  rsb = btt.tile([1, 512], bf16, tag="rsb", bufs=1)
                  nc.vector.tensor_copy(out=rsb, in_=rs)
                  pmu = psg.tile([P, 512], f32, tag="pb1")
                  nc.tensor.matmul(pmu, ones1b, mub, start=True, stop=True)
                  muB = btt.tile([P, 512], bf16, tag="muB", bufs=1)
                  nc.vector.tensor_copy(out=muB, in_=pmu)
                  prs = psg.tile([P, 512], f32, tag="pb1")
                  nc.tensor.matmul(prs, ones1b, rsb, start=True, stop=True)
                  rsB = btt.tile([P, 512], bf16, tag="rsB", bufs=1)
                  nc.vector.tensor_copy(out=rsB, in_=prs)

                  h_t = bt.tile([P, HC, 512], bf16, tag="h_t", bufs=2)
                  for hc in range(HC):
                      tmp = btt.tile([P, 512], bf16, tag="htmp", bufs=1)
                      nc.vector.tensor_tensor(out=tmp, in0=xbb[:, hc], in1=muB,
                                              op=OP.subtract)
                      nc.vector.tensor_tensor(out=h_t[:, hc], in0=tmp, in1=rsB,
                                              op=OP.mult)
                  kvl = bt.tile([P, LC, 512], bf16, tag="kvl")
                  for lc in range(LC):
                      pk = psg.tile([P, 512], f32, tag="pb1")
                      for hc in range(HC):
                          nc.tensor.matmul(pk, wkv_sb[:, hc, ts(lc, P)], h_t[:, hc],
                                           start=(hc == 0), stop=(hc == HC - 1))
                      nc.scalar.activation(out=kvl[:, lc], in_=pk, func=AF.Copy)
                  for i in range(4):
                      for hh in range(2):
                          pv = psg.tile([P, 512], f32, tag="pb1")
                          for lc in range(LC):
                              nc.tensor.matmul(pv, kvl[:, lc, ts(i, P)],
                                               wv_sb[:, lc, ts(hh, 512)],
                                               start=(lc == 0), stop=(lc == LC - 1))
                          nc.scalar.activation(out=v_sb[:, t * 4 + i, ts(hh, 512)],
                                               in_=pv, func=AF.Copy)
                  krf = bt.tile([P, HC, 512], bf16, tag="krf")
                  for hm in range(HC):
                      pk = psg.tile([P, 512], f32, tag="pb1")
                      for hc in range(HC):
                          nc.tensor.matmul(pk, wrk_sb[:, hc, ts(hm, P)], h_t[:, hc],
                                           start=(hc == 0), stop=(hc == HC - 1))
                      nc.vector.tensor_copy(out=krf[:, hm], in_=pk)
                  for c in range(HC):
                      cosc = btt.tile([P, 512], bf16, tag="cosc")
                      nc.sync.dma_start(out=cosc, in_=r128(cosb)[:, c, tsl])
                      sinc = btt.tile([P, 512], bf16, tag="sinc")
                      nc.sync.dma_start(out=sinc, in_=r128(sinb)[:, c, tsl])
                      t1 = btt.tile([P, 512], bf16, tag="kt1")
                      t2 = btt.tile([P, 512], bf16, tag="kt2")
                      nc.vector.tensor_tensor(out=t1, in0=krf[:, c], in1=cosc,
                                              op=OP.mult)
                      nc.vector.tensor_tensor(out=t2, in0=krf[:, (c + 4) % 8],
                                              in1=sinc, op=OP.mult)
                      nc.vector.tensor_tensor(out=krope[:, c, tsl], in0=t1, in1=t2,
                                              op=OP.subtract if c < 4 else OP.add)

          # ================== Attention ====================================
          with tc.tile_pool(name=f"cp{rep}", bufs=1) as cp, \
               tc.tile_pool(name=f"cpt{rep}", bufs=2) as cpt:
              mask_sb = cp.tile([P, KC, TT], bf16)
              nc.sync.dma_start(out=mask_sb, in_=msk.rearrange("k p q -> p k q"))
              e_sb = cp.tile([P, KC, TT], bf16)
              for kc in range(KC):
                  ps = psg.tile([P, TT], f32, tag="pb1")
                  for hc in range(HC):
                      nc.tensor.matmul(ps, krope[:, hc, ds(kc * P, P)], qrope[:, hc],
                                       start=(hc == 0), stop=(hc == HC - 1))
                  nc.scalar.activation(out=e_sb[:, kc], in_=ps, func=AF.Exp,
                                       scale=SCALE)
                  nc.vector.tensor_tensor(out=e_sb[:, kc], in0=e_sb[:, kc],
                                          in1=mask_sb[:, kc], op=OP.mult)
              pd = psg.tile([1, TT], f32, tag="pb1")
              for kc in range(KC):
                  nc.tensor.matmul(pd, ones128b, e_sb[:, kc],
                                   start=(kc == 0), stop=(kc == KC - 1))
              rden = cp.tile([1, TT], f32)
              nc.vector.reciprocal(out=rden, in_=pd)
              prb = psg.tile([P, TT], f32, tag="pb1")
              nc.tensor.matmul(prb, ones1f, rden, start=True, stop=True)
              rdenB = cp.tile([P, TT], f32)
              nc.vector.tensor_copy(out=rdenB, in_=prb)
              for hm in range(HC):
                  py = psg.tile([P, TT], f32, tag="pb1")
                  for kc in range(KC):
                      nc.tensor.matmul(py, v_sb[:, kc, ds(hm * P, P)], e_sb[:, kc],
                                       start=(kc == 0), stop=(kc == KC - 1))
                  nc.vector.tensor_tensor(out=yn[:, hm], in0=py, in1=rdenB,
                                          op=OP.mult)

          bw.release()   # frees attention weights
          # (bv still open: v/krope dead but space reclaimed below after cp closed)
          bv.release()

          # ================== o_proj + LN2 + router ========================
          d0 = tc.alloc_tile_pool(name=f"d0{rep}", bufs=1)
          h2f = d0.tile([P, HC, TT], f32)
          h2b = d0.tile([P, HC, TT], bf16)
          muB2 = d0.tile([P, TT], f32)
          rsB2 = d0.tile([P, TT], f32)
          probs = d0.tile([P, TT // P, 8], f32)
          nc.vector.memset(probs, -1e30)
          cmb = d0.tile([P, TT // P, 8], f32)
          nc.vector.memset(cmb, 0.0)
          h2nb = d0.tile([P, TT // P, H], bf16)

          with tc.tile_pool(name=f"d1{rep}", bufs=1) as d1, \
               tc.tile_pool(name=f"d1t{rep}", bufs=2) as d1t:
              wo_sb = d1.tile([P, HC, H], bf16)
              nc.sync.dma_start(out=wo_sb, in_=r128(woT))
              xon_sb = d1.tile([P, TT // P, H], f32)
              nc.sync.dma_start(out=xon_sb, in_=r128(xon))
              xoT_sb = d1.tile([P, HC, TT], f32)
              nc.sync.dma_start(out=xoT_sb, in_=r128(xoT))
              for tm in range(TT // P):
                  for hh in range(2):
                      po = psg.tile([P, 512], f32, tag="pb1")
                      for hc in range(HC):
                          nc.tensor.matmul(po, yn[:, hc, ts(tm, P)],
                                           wo_sb[:, hc, ts(hh, 512)],
                                           start=(hc == 0), stop=(hc == HC - 1))
                      nc.vector.tensor_tensor(out=xpn[:, tm, ts(hh, 512)], in0=po,
                                              in1=xon_sb[:, tm, ts(hh, 512)],
                                              op=OP.add)
              m2T = d1.tile([1, TT], f32)
              r2T = d1.tile([1, TT], f32)
              for tm in range(TT // P):
                  mr = _bn_mean_rstd(nc, d1t, xpn[:, tm], epsb128)
                  nc.vector.tensor_scalar(out=h2nb[:, tm], in0=xpn[:, tm],
                                          scalar1=mr[:, 0:1], scalar2=mr[:, 1:2],
                                          op0=OP.subtract, op1=OP.mult)
                  ptm = psg.tile([1, P], f32, tag="pb1")
                  nc.tensor.transpose(ptm, mr[:, 0:1], ident)
                  nc.vector.tensor_copy(out=m2T[:, ts(tm, P)], in_=ptm[0:1, :])
                  ptr = psg.tile([1, P], f32, tag="pb1")
                  nc.tensor.transpose(ptr, mr[:, 1:2], ident)
                  nc.vector.tensor_copy(out=r2T[:, ts(tm, P)], in_=ptr[0:1, :])
              pmu = psg.tile([P, TT], f32, tag="pb1")
              nc.tensor.matmul(pmu, ones1f, m2T, start=True, stop=True)
              nc.vector.tensor_copy(out=muB2, in_=pmu)
              prs = psg.tile([P, TT], f32, tag="pb1")
              nc.tensor.matmul(prs, ones1f, r2T, start=True, stop=True)
              nc.vector.tensor_copy(out=rsB2, in_=prs)

              for hm in range(HC):
                  po = psg.tile([P, TT], f32, tag="pb1")
                  for hc in range(HC):
                      nc.tensor.matmul(po, wo_sb[:, hc, ts(hm, P)], yn[:, hc],
                                       start=(hc == 0), stop=(hc == HC - 1))
                  xp = d1t.tile([P, TT], f32, tag="xp_t")
                  nc.vector.tensor_tensor(out=xp, in0=po, in1=xoT_sb[:, hm],
                                          op=OP.add)
                  tmp = d1t.tile([P, TT], f32, tag="h2tmp")
                  nc.vector.tensor_tensor(out=tmp, in0=xp, in1=muB2, op=OP.subtract)
                  nc.vector.tensor_tensor(out=h2f[:, hm], in0=tmp, in1=rsB2,
                                          op=OP.mult)
                  nc.scalar.activation(out=h2b[:, hm], in_=h2f[:, hm], func=AF.Copy)

              # router: fp32 matmuls, tokens on partitions
              for tm in range(TT // P):
                  pr = psg.tile([P, TT], f32, tag="pb1")
                  prr = pr[:, :E]
                  for hc in range(HC):
                      nc.tensor.matmul(prr, h2f[:, hc, ts(tm, P)], wrt_sb[:, hc],
                                       start=(hc == 0), stop=False)
                  nc.tensor.matmul(prr, ones1f, rbias_sb, start=False, stop=True)
                  nc.scalar.activation(out=probs[:, tm, :E], in_=prr, func=AF.Sigmoid)
                  top8 = d1t.tile([P, 8], f32, tag="top8")
                  nc.vector.max(out=top8, in_=probs[:, tm])
                  nc.vector.tensor_scalar(out=cmb[:, tm, :E], in0=probs[:, tm, :E],
                                          scalar1=top8[:, 1:2], scalar2=None,
                                          op0=OP.is_ge)
                  nc.vector.tensor_tensor(out=cmb[:, tm, :E], in0=cmb[:, tm, :E],
                                          in1=probs[:, tm, :E], op=OP.mult)

          psg.release()

          # ================== MoE: shared + 7 experts (gathered) ===========
          CAP = 256
          with tc.tile_pool(name=f"d2{rep}", bufs=1) as d2, \
               tc.tile_pool(name=f"d2w{rep}", bufs=2) as d2w, \
               tc.tile_pool(name=f"d2t{rep}", bufs=2) as d2t, \
               tc.tile_pool(name=f"psd{rep}", bufs=1, space="PSUM") as psd:
              identb = d2.tile([P, P], bf16)
              make_identity(nc, identb)
              moe = d2.tile([P, TT // P, H], f32)
              inter = d2.tile([P, FC, TT], bf16)

              # ---------- shared expert: dense over all 512 own tokens ------
              for fg in range(4):
                  wg = d2w.tile([P, HC, 512], bf16, tag="wg")
                  nc.sync.dma_start(out=wg, in_=r128(wsgT)[:, :, ts(fg, 512)])
                  wu = d2w.tile([P, HC, 512], bf16, tag="wu")
                  nc.sync.dma_start(out=wu, in_=r128(wsuT)[:, :, ts(fg, 512)])
                  for fs in range(4):
                      pg = psd.tile([P, TT], f32, tag="g", bufs=3)
                      pu = psd.tile([P, TT], f32, tag="u", bufs=2)
                      for hc in range(HC):
                          nc.tensor.matmul(pg, wg[:, hc, ts(fs, P)], h2b[:, hc],
                                           start=(hc == 0), stop=(hc == HC - 1))
                      for hc in range(HC):
                          nc.tensor.matmul(pu, wu[:, hc, ts(fs, P)], h2b[:, hc],
                                           start=(hc == 0), stop=(hc == HC - 1))
                      sg = d2t.tile([P, TT], f32, tag="sg")
                      nc.scalar.activation(out=sg, in_=pg, func=AF.Silu)
                      nc.vector.tensor_tensor(out=inter[:, fg * 4 + fs], in0=sg,
                                              in1=pu, op=OP.mult)
              for hh in range(2):
                  for half in range(2):
                      pdn = [psd.tile([P, TT], f32, tag=f"dn{i}", name=f"pdn{i}")
                             for i in range(2)]
                      for fc in range(FC):
                          wd = d2w.tile([P, 512], bf16, tag="wd", bufs=8)
                          nc.sync.dma_start(out=wd,
                                            in_=r128(wsdT)[:, fc, ts(hh, 512)])
                          for i in range(2):
                              tm = half * 2 + i
                              nc.tensor.matmul(pdn[i], inter[:, fc, ts(tm, P)], wd,
                                               start=(fc == 0), stop=(fc == FC - 1))
                      for i in range(2):
                          tm = half * 2 + i
                          nc.vector.tensor_copy(out=moe[:, tm, ts(hh, 512)],
                                                in_=pdn[i])

              # ---------- routed experts: gather cap=256 tokens each --------
              for ex in range(E):
                  inter_g = d2.tile([P, FC, CAP], bf16, tag="inter_g", bufs=2)
                  h2g = d2.tile([P, HC, CAP], bf16, tag="h2g", bufs=2)
                  SC = d2.tile([P, TT // P, CAP], bf16, tag="SC", bufs=2)
                  SCT = d2.tile([P, CAP // P, TT], bf16, tag="SCT", bufs=2)
                  y_eb = d2.tile([P, CAP // P, H], bf16, tag="y_eb", bufs=2)
                  # selection mask and prefix-sum positions
                  selb = d2t.tile([P, 4], bf16, tag="selb")
                  nc.gpsimd.tensor_scalar(out=selb, in0=cmb[:, :, ex],
                                          scalar1=0.0, scalar2=None, op0=OP.is_gt)
                  self32 = d2t.tile([P, 4], f32, tag="self32")
                  nc.gpsimd.tensor_scalar(out=self32, in0=cmb[:, :, ex],
                                          scalar1=0.0, scalar2=None, op0=OP.is_gt)
                  ppos = psd.tile([P, 2 * P], f32, tag="pp", bufs=1, name="ppos")[:, :4]
                  for tc4 in range(4):
                      for tcp in range(tc4 + 1):
                          blk = triS_sb if tcp == tc4 else ones2d_sb
                          nc.tensor.matmul(ppos[:, tc4:tc4 + 1], blk,
                                           selb[:, tcp:tcp + 1],
                                           start=(tcp == 0), stop=(tcp == tc4))
                  pos1 = d2t.tile([P, 4], f32, tag="pos1")
                  nc.vector.tensor_scalar(out=pos1, in0=ppos, scalar1=1.0,
                                          scalar2=None, op0=OP.add)
                  posm = d2t.tile([P, 4], f32, tag="posm")
                  nc.gpsimd.tensor_tensor(out=posm, in0=pos1, in1=self32,
                                          op=OP.mult)
                  posx = d2t.tile([P, 4], f32, tag="posx")
                  nc.gpsimd.tensor_scalar(out=posx, in0=posm, scalar1=-1.0,
                                          scalar2=None, op0=OP.add)
                  # one-hot gather matrix SC[t, g]; SCw = SC * combine weight;
                  # SCT = transpose(SCw) used for weighted scatter-add
                  SCw = d2.tile([P, TT // P, CAP], bf16, tag="SCw", bufs=2)
                  for tc4 in range(4):
                      nc.gpsimd.tensor_scalar(out=SC[:, tc4], in0=iob_sb,
                                              scalar1=posx[:, tc4:tc4 + 1],
                                              scalar2=None, op0=OP.is_equal)
                      nc.gpsimd.tensor_scalar(out=SCw[:, tc4], in0=SC[:, tc4],
                                              scalar1=cmb[:, tc4, ex:ex + 1],
                                              scalar2=None, op0=OP.mult)
                  for tc4 in range(4):
                      for gc in range(CAP // P):
                          ptt = psd.tile([P, 2 * P], bf16, tag="pp", bufs=1)
                          ptts = ptt[:, :P]
                          nc.tensor.transpose(ptts, SCw[:, tc4, ts(gc, P)], identb)
                          nc.scalar.activation(out=SCT[:, gc, ts(tc4, P)],
                                               in_=ptts, func=AF.Copy)
                  # gather h2 rows: h2g[h, g] = sum_t h2n[t, h] * SC[t, g]
                  for hm in range(HC):
                      pg2 = psd.tile([P, TT], f32, tag="g", bufs=3, name="pg2")[:, :CAP]
                      for tc4 in range(4):
                          nc.tensor.matmul(pg2, h2nb[:, tc4, ts(hm, P)], SC[:, tc4],
                                           start=(tc4 == 0), stop=(tc4 == 3))
                      nc.scalar.activation(out=h2g[:, hm], in_=pg2, func=AF.Copy)
                  # gate/up on gathered tokens
                  for fg in range(4):
                      wg = d2w.tile([P, HC, 512], bf16, tag="wg")
                      nc.sync.dma_start(out=wg,
                                        in_=r128(wegT[ex])[:, :, ts(fg, 512)])
                      wu = d2w.tile([P, HC, 512], bf16, tag="wu")
                      nc.sync.dma_start(out=wu,
                                        in_=r128(weuT[ex])[:, :, ts(fg, 512)])
                      for fs in range(4):
                          pg = psd.tile([P, TT], f32, tag="g", bufs=3)
                          pgs = pg[:, :CAP]
                          pu = psd.tile([P, TT], f32, tag="u", bufs=2)
                          pus = pu[:, :CAP]
                          for hc in range(HC):
                              nc.tensor.matmul(pgs, wg[:, hc, ts(fs, P)],
                                               h2g[:, hc],
                                               start=(hc == 0), stop=(hc == HC - 1))
                          for hc in range(HC):
                              nc.tensor.matmul(pus, wu[:, hc, ts(fs, P)],
                                               h2g[:, hc],
                                               start=(hc == 0), stop=(hc == HC - 1))
                          sg = d2t.tile([P, CAP], f32, tag="sgc")
                          nc.scalar.activation(out=sg, in_=pgs, func=AF.Silu)
                          nc.vector.tensor_tensor(out=inter_g[:, fg * 4 + fs],
                                                  in0=sg, in1=pus, op=OP.mult)
                  # down projection on gathered tokens -> y_eb [gtok, H]
                  for hh in range(2):
                      pdn = [psd.tile([P, TT], f32, tag=f"dn{i}", name=f"pdn{i}")
                             for i in range(CAP // P)]
                      for fc in range(FC):
                          wd = d2w.tile([P, 512], bf16, tag="wd", bufs=8)
                          nc.sync.dma_start(out=wd,
                                            in_=r128(wedT[ex])[:, fc, ts(hh, 512)])
                          for gm in range(CAP // P):
                              nc.tensor.matmul(pdn[gm][:, :512],
                                               inter_g[:, fc, ts(gm, P)], wd,
                                               start=(fc == 0), stop=(fc == FC - 1))
                      for gm in range(CAP // P):
                          nc.scalar.activation(out=y_eb[:, gm, ts(hh, 512)],
                                               in_=pdn[gm][:, :512], func=AF.Copy)
                  # scatter-add back: moe[t, h] += sum_g SCT[g, t-block] * y_eb[g, h]
                  for tm in range(TT // P):
                      for hh in range(2):
                          pm = psd.tile([P, TT], f32, tag="dn0", bufs=1, name="pm")
                          pms = pm[:, :512]
                          for gm in range(CAP // P):
                              nc.tensor.matmul(pms, SCT[:, gm, ts(tm, P)],
                                               y_eb[:, gm, ts(hh, 512)],
                                               start=(gm == 0),
                                               stop=(gm == CAP // P - 1))
                          nc.vector.tensor_tensor(out=moe[:, tm, ts(hh, 512)],
                                                  in0=moe[:, tm, ts(hh, 512)],
                                                  in1=pms, op=OP.add)

              for tm in range(TT // P):
                  nc.vector.tensor_tensor(out=xpn[:, tm], in0=xpn[:, tm],
                                          in1=moe[:, tm], op=OP.add)
                  nc.sync.dma_start(out=r128(out)[:, tm], in_=xpn[:, tm])

          d0.release()
          pp.release()
          cst.release()

    _split_multiwaits(nc)
    return nc


# ---------------------------------------------------------------------------
# Host side
# ---------------------------------------------------------------------------

_NC_CACHE = {}


def _get_nc(repeat=1):
    key = f"nc{repeat}"
    if key not in _NC_CACHE:
        _NC_CACHE[key] = build_nc(repeat)
    return _NC_CACHE[key]


def _rope_tables():
    inv_freq = 1.0 / (10000.0 ** (np.arange(0, H, 2, dtype=np.float64) / H))
    t = np.arange(T, dtype=np.float64)
    freqs = np.outer(t, inv_freq)
    emb = np.concatenate([freqs, freqs], axis=-1)          # [T, H]
    return (np.cos(emb).astype(np.float32).T.copy(),
            np.sin(emb).astype(np.float32).T.copy())       # [H, T]


def make_in_maps(inputs):
    bf = ml_dtypes.bfloat16
    x = np.asarray(inputs["x"], np.float32)
    ln1 = np.asarray(inputs["ln1_w"], np.float32)
    ln2 = np.asarray(inputs["ln2_w"], np.float32)

    def tb(a):  # transpose last two dims, contiguous, bf16
        return np.ascontiguousarray(np.swapaxes(a, -1, -2)).astype(bf)

    wkvT = tb(np.asarray(inputs["kv_proj_d"]) * ln1[None, :])
    wqT = tb(np.asarray(inputs["q_proj_d"]) * ln1[None, :])
    wrkT = tb(np.asarray(inputs["rope_k"]) * ln1[None, :])
    wvT = tb(np.asarray(inputs["v_proj_u"]))
    wrqT = tb(np.asarray(inputs["rope_q"]))
    woT = tb(np.asarray(inputs["o_proj"]))
    wrtT = np.ascontiguousarray(
        (np.asarray(inputs["router_w"], np.float32) * ln2[None, :]).T
        .reshape(HC, P, E).transpose(1, 0, 2))
    rbias = np.asarray(inputs["routing_bias"], np.float32).reshape(1, E)
    wsgT = tb(np.asarray(inputs["sh_gate"]) * ln2[None, :])
    wsuT = tb(np.asarray(inputs["sh_up"]) * ln2[None, :])
    wsdT = tb(np.asarray(inputs["sh_down"]))
    wegT = tb(np.asarray(inputs["ex_gate"]) * ln2[None, None, :])
    weuT = tb(np.asarray(inputs["ex_up"]) * ln2[None, None, :])
    wedT = tb(np.asarray(inputs["ex_down"]))

    cosT, sinT = _rope_tables()
    cosb = cosT.astype(bf)
    sinb = sinT.astype(bf)

    xT = np.ascontiguousarray(x.transpose(0, 2, 1))  # [B, H, T]
    iob_np = np.tile(np.arange(256, dtype=np.float32), (P, 1))
    triS_np = np.tril(np.ones((P, P), np.float32), -1).astype(bf)
    ones2d_np = np.ones((P, P), np.float32).astype(bf)

    in_maps = []
    for c in range(N_CORES):
        b, j = c // 4, c % 4
        qoff = 512 * j
        kk = np.arange(TB).reshape(KC, P, 1)
        qq = qoff + np.arange(TT).reshape(1, 1, TT)
        msk = (kk <= qq).astype(bf)
        in_maps.append({
            "xbT": xT[b],
            "xoT": np.ascontiguousarray(xT[b][:, qoff:qoff + TT]),
            "xon": np.ascontiguousarray(x[b][qoff:qoff + TT, :]),
            "cosb": cosb, "sinb": sinb,
            "cosq": np.ascontiguousarray(cosb[:, qoff:qoff + TT]),
            "sinq": np.ascontiguousarray(sinb[:, qoff:qoff + TT]),
            "msk": msk,
            "wkvT": wkvT, "wqT": wqT, "wvT": wvT, "wrqT": wrqT,
            "wrkT": wrkT, "woT": woT, "wrtT": wrtT, "rbias": rbias,
            "wsgT": wsgT, "wsuT": wsuT, "wsdT": wsdT,
            "wegT": wegT, "weuT": weuT, "wedT": wedT,
            "iob": iob_np, "triS": triS_np, "ones2d": ones2d_np,
        })
    return in_maps


def kernel(**inputs):
    in_maps = make_in_maps(inputs)
    import os
    nc = _get_nc()
    trace = bool(int(os.environ.get("KERNEL_TRACE", "0")))
    res = run_bass_kernel_spmd(nc, in_maps, core_ids=list(range(N_CORES)),
                               trace=trace,
                               trace_cores=[0, 3, 7] if trace else None)
    _NC_CACHE["last_result"] = res

    outp = np.empty((B, T, H), np.float32)
    for c in range(N_CORES):
        b, j = c // 4, c % 4
        outp[b, 512 * j:512 * (j + 1), :] = res.results[c]["out"]
    return outp



# revision 24
# speedup vs baseline: 1.4610x; 1.4610x over previous
"""DeepSeek block (MLA attention + shared MLP + 7-expert top-2 MoE) on 8 TRN2
NeuronCores.

Sharding: core c handles batch b=c//4, query block j=c%4 (512 tokens) for
attention/MoE; K/V for the full 2048-token batch slab are computed redundantly
on each of the 4 cores of a batch group (uniform SPMD program, no collectives).
Causality is enforced by 0/1 value masks supplied per core.

Layouts: activations live as [128 partitions = H%128, H//128 chunks, tokens]
("T-layout") so every matmul contraction is on partitions; all weights are
pre-transposed on the host. The MoE down-projection emits [tokens, H] so the
per-token top-2 combine weight is a native per-partition scalar.
"""

import functools

import numpy as np
import ml_dtypes

import concourse.bass as bass
import concourse.tile as tile
from concourse import mybir
from concourse.bass import ds, ts
from concourse.bass_utils import run_bass_kernel_spmd
from concourse.masks import make_identity

f32 = mybir.dt.float32
bf16 = mybir.dt.bfloat16
fp8 = mybir.dt.float8e4
AF = mybir.ActivationFunctionType
OP = mybir.AluOpType
PM = mybir.MatmulPerfMode

# fp8 quantization scales: device activations are stored as value*C_H etc.
C_H = 16.0   # h2 (layernorm output, |x| <~ 5)
C_I = 8.0    # MLP intermediate silu(g)*u, |x| <~ 10

P = 128
B, T, H, L, F, E = 2, 2048, 1024, 256, 2048, 7
HC, LC, FC = H // P, L // P, F // P  # 8, 2, 16
TT = 512          # own tokens per core
TB = 2048         # batch slab tokens
NTB = TB // 512   # 4 batch token tiles
KC = TB // P      # 16 key chunks
EPS = 1e-5
SCALE = 1.0 / 32.0  # 1/sqrt(H)
N_CORES = 8


def _split_multiwaits(nc, max_waits=1):
    """walrus here supports one sync-wait per instruction; hoist extras onto
    preceding NoOps on the same engine."""
    ctr = 0
    for f in nc.m.functions:
        for bb in f.blocks:
            out = []
            dirty = False
            for inst in bb.instructions:
                si = inst.sync_info
                if si is not None and len(si.on_wait) > max_waits:
                    waits = list(si.on_wait)
                    for w in waits[:-max_waits]:
                        ctr += 1
                        nop = mybir.InstNoOp(name=f"waitnop-{ctr}", ins=[], outs=[])
                        nop.engine = inst.engine
                        nop.sync_info = mybir.SyncInfo(on_wait=[w], on_update=[])
                        out.append(nop)
                    inst.sync_info = mybir.SyncInfo(
                        on_wait=waits[-max_waits:], on_update=list(si.on_update)
                    )
                    dirty = True
                out.append(inst)
            if dirty:
                bb.instructions = out
    return ctr


def _bn_mean_rstd(nc, pool, src_ap, epsb):
    """src_ap [128, 1024] f32 -> mr [128, 2] (mean, rstd) via bn_stats."""
    stats = pool.tile([P, 2, 6], f32, tag="bn_stats")
    nc.vector.bn_stats(out=stats[:, 0], in_=src_ap[:, 0:512])
    nc.vector.bn_stats(out=stats[:, 1], in_=src_ap[:, 512:1024])
    mv = pool.tile([P, 2], f32, tag="bn_mv")
    nc.vector.bn_aggr(out=mv, in_=stats)
    mr = pool.tile([P, 2], f32, tag="bn_mr")
    nc.vector.tensor_copy(out=mr[:, 0:1], in_=mv[:, 0:1])
    nc.scalar.activation(out=mr[:, 1:2], in_=mv[:, 1:2], func=AF.Sqrt, bias=epsb)
    nc.vector.reciprocal(out=mr[:, 1:2], in_=mr[:, 1:2])
    return mr


def build_nc(repeat=1, scales=None):
    # scales: host-side fp8 weight quant factors (w_q = w * F); dequant is
    # folded into the activation/copy ops after each PSUM group.
    sc = scales or {}
    dq_eg = [1.0 / (f * C_H) for f in sc["eg"]]
    dq_eu_i = [C_I / (f * C_H) for f in sc["eu"]]
    dq_ed = [1.0 / (f * C_I) for f in sc["ed"]]

    nc = bass.Bass()

    def din(name, shape, dt=bf16):
        return nc.declare_dram_parameter(name, list(shape), dt, isOutput=False)

    xbT = din("xbT", [H, TB], f32)
    xoT = din("xoT", [H, TT], f32)
    xon = din("xon", [TT, H], f32)
    cosb = din("cosb", [H, TB])
    sinb = din("sinb", [H, TB])
    cosq = din("cosq", [H, TT])
    sinq = din("sinq", [H, TT])
    msk = din("msk", [KC, P, TT])
    wkvT = din("wkvT", [H, L])
    wqT = din("wqT", [H, L])
    wvT = din("wvT", [L, H])
    wrqT = din("wrqT", [L, H])
    wrkT = din("wrkT", [H, H])
    woT = din("woT", [H, H])
    wrtT = din("wrtT", [P, HC, E], f32)
    rbias = din("rbias", [1, E], f32)
    wsgT = din("wsgT", [H, F])
    wsuT = din("wsuT", [H, F])
    wsdT = din("wsdT", [F, H])
    iob = din("iob", [P, 256], f32)
    triS = din("triS", [P, P])
    ones2d = din("ones2d", [P, P])
    wegT = din("wegT", [E, H, F], fp8)
    weuT = din("weuT", [E, H, F], fp8)
    wedT = din("wedT", [E, F, H], fp8)
    out = nc.declare_dram_parameter("out", [TT, H], f32, isOutput=True)

    r128 = lambda ap: ap.rearrange("(c p) x -> p c x", p=P)

    with tile.TileContext(nc) as tc:
      for rep in range(repeat):
          cst = tc.alloc_tile_pool(name=f"cst{rep}", bufs=1)
          pp = tc.alloc_tile_pool(name=f"pp{rep}", bufs=1)       # persist: qrope, yn, xpn
          psg = tc.alloc_tile_pool(name=f"psg{rep}", bufs=4, space="PSUM")

          ones128b = cst.tile([P, 1], bf16)
          nc.vector.memset(ones128b, 1.0)
          ones1b = cst.tile([1, P], bf16)
          nc.vector.memset(ones1b, 1.0)
          ones1f = cst.tile([1, P], f32)
          nc.vector.memset(ones1f, 1.0)
          epsb1 = cst.tile([1, 1], f32)
          nc.vector.memset(epsb1, EPS)
          epsb128 = cst.tile([P, 1], f32)
          nc.vector.memset(epsb128, EPS)
          ident = cst.tile([P, P], f32)
          make_identity(nc, ident)
          wrt_sb = cst.tile([P, HC, E], f32)
          nc.sync.dma_start(out=wrt_sb, in_=wrtT[:, :, :])
          rbias_sb = cst.tile([1, E], f32)
          nc.sync.dma_start(out=rbias_sb, in_=rbias[:, :])
          iob_sb = cst.tile([P, 256], f32)
          nc.sync.dma_start(out=iob_sb, in_=iob[:, :])
          triS_sb = cst.tile([P, P], bf16)
          nc.sync.dma_start(out=triS_sb, in_=triS[:, :])
          ones2d_sb = cst.tile([P, P], bf16)
          nc.sync.dma_start(out=ones2d_sb, in_=ones2d[:, :])

          qrope = pp.tile([P, HC, TT], bf16)
          yn = pp.tile([P, HC, TT], bf16)
          xpn = pp.tile([P, TT // P, H], f32)

          bv = tc.alloc_tile_pool(name=f"bv{rep}", bufs=1)
          v_sb = bv.tile([P, KC, H], bf16)
          krope = bv.tile([P, HC, TB], bf16)

          bw = tc.alloc_tile_pool(name=f"bw{rep}", bufs=1)
          wkv_sb = bw.tile([P, HC, L], bf16)
          nc.sync.dma_start(out=wkv_sb, in_=r128(wkvT))
          wq_sb = bw.tile([P, HC, L], bf16)
          nc.sync.dma_start(out=wq_sb, in_=r128(wqT))
          wv_sb = bw.tile([P, LC, H], bf16)
          nc.sync.dma_start(out=wv_sb, in_=r128(wvT))
          wrq_sb = bw.tile([P, LC, H], bf16)
          nc.sync.dma_start(out=wrq_sb, in_=r128(wrqT))
          wrk_sb = bw.tile([P, HC, H], bf16)
          nc.sync.dma_start(out=wrk_sb, in_=r128(wrkT))

          # ================== OWN pipeline: h_own -> q_lat -> qrope =========
          with tc.tile_pool(name=f"own{rep}", bufs=1) as own, \
               tc.tile_pool(name=f"ownt{rep}", bufs=2) as ownt:
              muT = own.tile([1, TT], f32)
              rsT = own.tile([1, TT], f32)
              for tm in range(TT // P):
                  xon_t = ownt.tile([P, H], f32, tag="xon_t")
                  nc.sync.dma_start(out=xon_t, in_=r128(xon)[:, tm])
                  mr = _bn_mean_rstd(nc, ownt, xon_t, epsb128)
                  ptm = psg.tile([1, P], f32, tag="pb1")
                  nc.tensor.transpose(ptm, mr[:, 0:1], ident)
                  nc.vector.tensor_copy(out=muT[:, ts(tm, P)], in_=ptm[0:1, :])
                  ptr = psg.tile([1, P], f32, tag="pb1")
                  nc.tensor.transpose(ptr, mr[:, 1:2], ident)
                  nc.vector.tensor_copy(out=rsT[:, ts(tm, P)], in_=ptr[0:1, :])
              pmu = psg.tile([P, TT], f32, tag="pb1")
              nc.tensor.matmul(pmu, ones1f, muT, start=True, stop=True)
              muB = own.tile([P, TT], f32)
              nc.vector.tensor_copy(out=muB, in_=pmu)
              prs = psg.tile([P, TT], f32, tag="pb1")
              nc.tensor.matmul(prs, ones1f, rsT, start=True, stop=True)
              rsB = own.tile([P, TT], f32)
              nc.vector.tensor_copy(out=rsB, in_=prs)

              h_own = own.tile([P, HC, TT], bf16)
              for hc in range(HC):
                  xoT_t = ownt.tile([P, TT], f32, tag="xoT_t")
                  nc.sync.dma_start(out=xoT_t, in_=r128(xoT)[:, hc])
                  tmp = ownt.tile([P, TT], f32, tag="ot_f32")
                  nc.vector.tensor_tensor(out=tmp, in0=xoT_t, in1=muB,
                                          op=OP.subtract)
                  nc.vector.tensor_tensor(out=h_own[:, hc], in0=tmp, in1=rsB,
                                          op=OP.mult)
              qlat = own.tile([P, LC, TT], bf16)
              for lc in range(LC):
                  pq = psg.tile([P, TT], f32, tag="pb1")
                  for hc in range(HC):
                      nc.tensor.matmul(pq, wq_sb[:, hc, ts(lc, P)], h_own[:, hc],
                                       start=(hc == 0), stop=(hc == HC - 1))
                  nc.scalar.activation(out=qlat[:, lc], in_=pq, func=AF.Copy)
              qrf = own.tile([P, HC, TT], bf16)
              for hm in range(HC):
                  pq = psg.tile([P, TT], f32, tag="pb1")
                  for lc in range(LC):
                      nc.tensor.matmul(pq, wrq_sb[:, lc, ts(hm, P)], qlat[:, lc],
                                       start=(lc == 0), stop=(lc == LC - 1))
                  nc.scalar.activation(out=qrf[:, hm], in_=pq, func=AF.Copy)
              cosq_sb = own.tile([P, HC, TT], bf16)
              nc.sync.dma_start(out=cosq_sb, in_=r128(cosq))
              sinq_sb = own.tile([P, HC, TT], bf16)
              nc.sync.dma_start(out=sinq_sb, in_=r128(sinq))
              for c in range(HC):
                  t1 = ownt.tile([P, TT], bf16, tag="ot_b1")
                  t2 = ownt.tile([P, TT], bf16, tag="ot_b2")
                  nc.vector.tensor_tensor(out=t1, in0=qrf[:, c], in1=cosq_sb[:, c],
                                          op=OP.mult)
                  nc.vector.tensor_tensor(out=t2, in0=qrf[:, (c + 4) % 8],
                                          in1=sinq_sb[:, c], op=OP.mult)
                  nc.vector.tensor_tensor(out=qrope[:, c], in0=t1, in1=t2,
                                          op=OP.subtract if c < 4 else OP.add)

          # ================== BATCH loop: h, kv_lat, v, krope ===============
          with tc.tile_pool(name=f"bst{rep}", bufs=1, space="PSUM") as bst, \
               tc.tile_pool(name=f"bt{rep}", bufs=1) as bt, \
               tc.tile_pool(name=f"btt{rep}", bufs=2) as btt:
              for t in range(NTB):
                  tsl = ts(t, 512)
                  xbb = bt.tile([P, HC, 512], bf16, tag="xbb", bufs=2)
                  psA = bst.tile([1, 512], f32, tag="psA")
                  psB = bst.tile([1, 512], f32, tag="psB")
                  for hc in range(HC):
                      xbh = btt.tile([P, 512], f32, tag="xbh", bufs=3)
                      nc.sync.dma_start(out=xbh, in_=r128(xbT)[:, hc, tsl])
                      nc.scalar.activation(out=xbb[:, hc], in_=xbh, func=AF.Copy)
                      xsq = btt.tile([P, 512], bf16, tag="xsq", bufs=1)
                      nc.vector.tensor_tensor(out=xsq, in0=xbb[:, hc],
                                              in1=xbb[:, hc], op=OP.mult)
                      nc.tensor.matmul(psA, ones128b, xbb[:, hc],
                                       start=(hc == 0), stop=(hc == HC - 1))
                      nc.tensor.matmul(psB, ones128b, xsq,
                                       start=(hc == 0), stop=(hc == HC - 1))
                  mu = btt.tile([1, 512], f32, tag="mu", bufs=1)
                  nc.vector.tensor_scalar_mul(out=mu, in0=psA, scalar1=1.0 / H)
                  mu2 = btt.tile([1, 512], f32, tag="mu2", bufs=1)
                  nc.vector.tensor_tensor(out=mu2, in0=mu, in1=mu, op=OP.mult)
                  var = btt.tile([1, 512], f32, tag="var", bufs=1)
                  nc.vector.scalar_tensor_tensor(out=var, in0=psB, scalar=1.0 / H,
                                                 in1=mu2, op0=OP.mult, op1=OP.subtract)
                  nc.scalar.activation(out=var, in_=var, func=AF.Sqrt, bias=epsb1)
                  rs = var
                  nc.vector.reciprocal(out=rs, in_=rs)
                  mub = btt.tile([1, 512], bf16, tag="mub", bufs=1)
                  nc.vector.tensor_copy(out=mub, in_=mu)
                  rsb = btt.tile([1, 512], bf16, tag="rsb", bufs=1)
                  nc.vector.tensor_copy(out=rsb, in_=rs)
                  pmu = psg.tile([P, 512], f32, tag="pb1")
                  nc.tensor.matmul(pmu, ones1b, mub, start=True, stop=True)
                  muB = btt.tile([P, 512], bf16, tag="muB", bufs=1)
                  nc.vector.tensor_copy(out=muB, in_=pmu)
                  prs = psg.tile([P, 512], f32, tag="pb1")
                  nc.tensor.matmul(prs, ones1b, rsb, start=True, stop=True)
                  rsB = btt.tile([P, 512], bf16, tag="rsB", bufs=1)
                  nc.vector.tensor_copy(out=rsB, in_=prs)

                  h_t = bt.tile([P, HC, 512], bf16, tag="h_t", bufs=2)
                  for hc in range(HC):
                      tmp = btt.tile([P, 512], bf16, tag="htmp", bufs=1)
                      nc.vector.tensor_tensor(out=tmp, in0=xbb[:, hc], in1=muB,
                                              op=OP.subtract)
                      nc.vector.tensor_tensor(out=h_t[:, hc], in0=tmp, in1=rsB,
                                              op=OP.mult)
                  kvl = bt.tile([P, LC, 512], bf16, tag="kvl")
                  for lc in range(LC):
                      pk = psg.tile([P, 512], f32, tag="pb1")
                      for hc in range(HC):
                          nc.tensor.matmul(pk, wkv_sb[:, hc, ts(lc, P)], h_t[:, hc],
                                           start=(hc == 0), stop=(hc == HC - 1))
                      nc.scalar.activation(out=kvl[:, lc], in_=pk, func=AF.Copy)
                  for i in range(4):
                      for hh in range(2):
                          pv = psg.tile([P, 512], f32, tag="pb1")
                          for lc in range(LC):
                              nc.tensor.matmul(pv, kvl[:, lc, ts(i, P)],
                                               wv_sb[:, lc, ts(hh, 512)],
                                               start=(lc == 0), stop=(lc == LC - 1))
                          nc.scalar.activation(out=v_sb[:, t * 4 + i, ts(hh, 512)],
                                               in_=pv, func=AF.Copy)
                  krf = bt.tile([P, HC, 512], bf16, tag="krf")
                  for hm in range(HC):
                      pk = psg.tile([P, 512], f32, tag="pb1")
                      for hc in range(HC):
                          nc.tensor.matmul(pk, wrk_sb[:, hc, ts(hm, P)], h_t[:, hc],
                                           start=(hc == 0), stop=(hc == HC - 1))
                      nc.vector.tensor_copy(out=krf[:, hm], in_=pk)
                  for c in range(HC):
                      cosc = btt.tile([P, 512], bf16, tag="cosc")
                      nc.sync.dma_start(out=cosc, in_=r128(cosb)[:, c, tsl])
                      sinc = btt.tile([P, 512], bf16, tag="sinc")
                      nc.sync.dma_start(out=sinc, in_=r128(sinb)[:, c, tsl])
                      t1 = btt.tile([P, 512], bf16, tag="kt1")
                      t2 = btt.tile([P, 512], bf16, tag="kt2")
                      nc.vector.tensor_tensor(out=t1, in0=krf[:, c], in1=cosc,
                                              op=OP.mult)
                      nc.vector.tensor_tensor(out=t2, in0=krf[:, (c + 4) % 8],
                                              in1=sinc, op=OP.mult)
                      nc.vector.tensor_tensor(out=krope[:, c, tsl], in0=t1, in1=t2,
                                              op=OP.subtract if c < 4 else OP.add)

          # ================== Attention ====================================
          with tc.tile_pool(name=f"cp{rep}", bufs=1) as cp, \
               tc.tile_pool(name=f"cpt{rep}", bufs=2) as cpt:
              mask_sb = cp.tile([P, KC, TT], bf16)
              nc.sync.dma_start(out=mask_sb, in_=msk.rearrange("k p q -> p k q"))
              e_sb = cp.tile([P, KC, TT], bf16)
              for kc in range(KC):
                  ps = psg.tile([P, TT], f32, tag="pb1")
                  for hc in range(HC):
                      nc.tensor.matmul(ps, krope[:, hc, ds(kc * P, P)], qrope[:, hc],
                                       start=(hc == 0), stop=(hc == HC - 1))
                  nc.scalar.activation(out=e_sb[:, kc], in_=ps, func=AF.Exp,
                                       scale=SCALE)
                  nc.vector.tensor_tensor(out=e_sb[:, kc], in0=e_sb[:, kc],
                                          in1=mask_sb[:, kc], op=OP.mult)
              pd = psg.tile([1, TT], f32, tag="pb1")
              for kc in range(KC):
                  nc.tensor.matmul(pd, ones128b, e_sb[:, kc],
                                   start=(kc == 0), stop=(kc == KC - 1))
              rden = cp.tile([1, TT], f32)
              nc.vector.reciprocal(out=rden, in_=pd)
              prb = psg.tile([P, TT], f32, tag="pb1")
              nc.tensor.matmul(prb, ones1f, rden, start=True, stop=True)
              rdenB = cp.tile([P, TT], f32)
              nc.vector.tensor_copy(out=rdenB, in_=prb)
              for hm in range(HC):
                  py = psg.tile([P, TT], f32, tag="pb1")
                  for kc in range(KC):
                      nc.tensor.matmul(py, v_sb[:, kc, ds(hm * P, P)], e_sb[:, kc],
                                       start=(kc == 0), stop=(kc == KC - 1))
                  nc.vector.tensor_tensor(out=yn[:, hm], in0=py, in1=rdenB,
                                          op=OP.mult)

          bw.release()   # frees attention weights
          # (bv still open: v/krope dead but space reclaimed below after cp closed)
          bv.release()

          # ================== o_proj + LN2 + router ========================
          d0 = tc.alloc_tile_pool(name=f"d0{rep}", bufs=1)
          h2f = d0.tile([P, HC, TT], f32)
          h2b = d0.tile([P, HC, TT], bf16)
          muB2 = d0.tile([P, TT], f32)
          rsB2 = d0.tile([P, TT], f32)
          probs = d0.tile([P, TT // P, 8], f32)
          nc.vector.memset(probs, -1e30)
          cmb = d0.tile([P, TT // P, 8], f32)
          nc.vector.memset(cmb, 0.0)
          h2nb = d0.tile([P, TT // P, H], fp8)

          with tc.tile_pool(name=f"d1{rep}", bufs=1) as d1, \
               tc.tile_pool(name=f"d1t{rep}", bufs=2) as d1t:
              wo_sb = d1.tile([P, HC, H], bf16)
              nc.sync.dma_start(out=wo_sb, in_=r128(woT))
              xon_sb = d1.tile([P, TT // P, H], f32)
              nc.sync.dma_start(out=xon_sb, in_=r128(xon))
              xoT_sb = d1.tile([P, HC, TT], f32)
              nc.sync.dma_start(out=xoT_sb, in_=r128(xoT))
              for tm in range(TT // P):
                  for hh in range(2):
                      po = psg.tile([P, 512], f32, tag="pb1")
                      for hc in range(HC):
                          nc.tensor.matmul(po, yn[:, hc, ts(tm, P)],
                                           wo_sb[:, hc, ts(hh, 512)],
                                           start=(hc == 0), stop=(hc == HC - 1))
                      nc.vector.tensor_tensor(out=xpn[:, tm, ts(hh, 512)], in0=po,
                                              in1=xon_sb[:, tm, ts(hh, 512)],
                                              op=OP.add)
              m2T = d1.tile([1, TT], f32)
              r2T = d1.tile([1, TT], f32)
              for tm in range(TT // P):
                  mr = _bn_mean_rstd(nc, d1t, xpn[:, tm], epsb128)
                  mrc = d1t.tile([P, 1], f32, tag="mrc")
                  nc.vector.tensor_scalar_mul(out=mrc, in0=mr[:, 1:2],
                                              scalar1=C_H)
                  nc.vector.tensor_scalar(out=h2nb[:, tm], in0=xpn[:, tm],
                                          scalar1=mr[:, 0:1], scalar2=mrc,
                                          op0=OP.subtract, op1=OP.mult)
                  ptm = psg.tile([1, P], f32, tag="pb1")
                  nc.tensor.transpose(ptm, mr[:, 0:1], ident)
                  nc.vector.tensor_copy(out=m2T[:, ts(tm, P)], in_=ptm[0:1, :])
                  ptr = psg.tile([1, P], f32, tag="pb1")
                  nc.tensor.transpose(ptr, mr[:, 1:2], ident)
                  nc.vector.tensor_copy(out=r2T[:, ts(tm, P)], in_=ptr[0:1, :])
              pmu = psg.tile([P, TT], f32, tag="pb1")
              nc.tensor.matmul(pmu, ones1f, m2T, start=True, stop=True)
              nc.vector.tensor_copy(out=muB2, in_=pmu)
              prs = psg.tile([P, TT], f32, tag="pb1")
              nc.tensor.matmul(prs, ones1f, r2T, start=True, stop=True)
              nc.vector.tensor_copy(out=rsB2, in_=prs)

              for hm in range(HC):
                  po = psg.tile([P, TT], f32, tag="pb1")
                  for hc in range(HC):
                      nc.tensor.matmul(po, wo_sb[:, hc, ts(hm, P)], yn[:, hc],
                                       start=(hc == 0), stop=(hc == HC - 1))
                  xp = d1t.tile([P, TT], f32, tag="xp_t")
                  nc.vector.tensor_tensor(out=xp, in0=po, in1=xoT_sb[:, hm],
                                          op=OP.add)
                  tmp = d1t.tile([P, TT], f32, tag="h2tmp")
                  nc.vector.tensor_tensor(out=tmp, in0=xp, in1=muB2, op=OP.subtract)
                  nc.vector.tensor_tensor(out=h2f[:, hm], in0=tmp, in1=rsB2,
                                          op=OP.mult)
                  nc.scalar.activation(out=h2b[:, hm], in_=h2f[:, hm],
                                       func=AF.Copy)

              # router: fp32 matmuls, tokens on partitions
              for tm in range(TT // P):
                  pr = psg.tile([P, TT], f32, tag="pb1")
                  prr = pr[:, :E]
                  for hc in range(HC):
                      nc.tensor.matmul(prr, h2f[:, hc, ts(tm, P)], wrt_sb[:, hc],
                                       start=(hc == 0), stop=False)
                  nc.tensor.matmul(prr, ones1f, rbias_sb, start=False, stop=True)
                  nc.scalar.activation(out=probs[:, tm, :E], in_=prr, func=AF.Sigmoid)
                  top8 = d1t.tile([P, 8], f32, tag="top8")
                  nc.vector.max(out=top8, in_=probs[:, tm])
                  nc.vector.tensor_scalar(out=cmb[:, tm, :E], in0=probs[:, tm, :E],
                                          scalar1=top8[:, 1:2], scalar2=None,
                                          op0=OP.is_ge)
                  nc.vector.tensor_tensor(out=cmb[:, tm, :E], in0=cmb[:, tm, :E],
                                          in1=probs[:, tm, :E], op=OP.mult)

          psg.release()

          # ================== MoE: shared + 7 experts (gathered) ===========
          CAP = 256
          with tc.tile_pool(name=f"d2{rep}", bufs=1) as d2, \
               tc.tile_pool(name=f"d2w{rep}", bufs=2) as d2w, \
               tc.tile_pool(name=f"d2t{rep}", bufs=2) as d2t, \
               tc.tile_pool(name=f"psd{rep}", bufs=1, space="PSUM") as psd:
              identb = d2.tile([P, P], bf16)
              make_identity(nc, identb)
              moe = d2.tile([P, TT // P, H], f32)
              inter = d2.tile([P, FC, TT], bf16)

              # ---------- routed-expert gather matrices, hoisted ------------
              # Selection + prefix-sum batched across all 7 experts, then
              # per-expert one-hot SC/SCw on gpsimd -- all issued before the
              # shared expert so gpsimd runs ahead of the routed matmul
              # stream (was ~12us of tensor idle per expert).
              SCl = [d2.tile([P, TT // P, CAP], fp8, tag=f"SCl{e}",
                             name=f"SCl{e}") for e in range(E)]
              SCwl = [d2.tile([P, TT // P, CAP], bf16, tag=f"SCwl{e}",
                              name=f"SCwl{e}") for e in range(E)]
              selb = d2t.tile([P, 4, E], bf16, tag="selb")
              nc.gpsimd.tensor_scalar(out=selb, in0=cmb[:, :, :E],
                                      scalar1=0.0, scalar2=None, op0=OP.is_gt)
              self32 = d2t.tile([P, 4, E], f32, tag="self32")
              nc.gpsimd.tensor_scalar(out=self32, in0=cmb[:, :, :E],
                                      scalar1=0.0, scalar2=None, op0=OP.is_gt)
              ppos = psd.tile([P, 2 * P], f32, tag="pp", bufs=1,
                              name="ppos").rearrange("p (a b) -> p a b",
                                                     a=4)[:, :, :E]
              for tc4 in range(4):
                  for tcp in range(tc4 + 1):
                      blk = triS_sb if tcp == tc4 else ones2d_sb
                      nc.tensor.matmul(ppos[:, tc4], blk, selb[:, tcp],
                                       start=(tcp == 0), stop=(tcp == tc4))
              posx = d2t.tile([P, 4, E], f32, tag="posx")
              nc.vector.tensor_scalar(out=posx, in0=ppos, scalar1=1.0,
                                      scalar2=None, op0=OP.add)
              nc.gpsimd.tensor_tensor(out=posx, in0=posx, in1=self32,
                                      op=OP.mult)
              nc.gpsimd.tensor_scalar(out=posx, in0=posx, scalar1=-1.0,
                                      scalar2=None, op0=OP.add)
              for ex in range(E):
                  for tc4 in range(4):
                      nc.gpsimd.tensor_scalar(out=SCl[ex][:, tc4], in0=iob_sb,
                                              scalar1=posx[:, tc4, ex:ex + 1],
                                              scalar2=None, op0=OP.is_equal)
                      nc.gpsimd.tensor_scalar(out=SCwl[ex][:, tc4],
                                              in0=iob_sb,
                                              scalar1=posx[:, tc4, ex:ex + 1],
                                              scalar2=cmb[:, tc4, ex:ex + 1],
                                              op0=OP.is_equal, op1=OP.mult)

              # ---------- shared expert: dense over all 512 own tokens ------
              for fg in range(4):
                  wg = d2w.tile([P, HC, 512], bf16, tag="wg")
                  nc.sync.dma_start(out=wg, in_=r128(wsgT)[:, :, ts(fg, 512)])
                  wu = d2w.tile([P, HC, 512], bf16, tag="wu")
                  nc.sync.dma_start(out=wu, in_=r128(wsuT)[:, :, ts(fg, 512)])
                  for fs in range(4):
                      pg = psd.tile([P, TT], f32, tag="g", bufs=3)
                      pu = psd.tile([P, TT], f32, tag="u", bufs=2)
                      for hc in range(HC):
                          nc.tensor.matmul(pg, wg[:, hc, ts(fs, P)], h2b[:, hc],
                                           start=(hc == 0), stop=(hc == HC - 1))
                      for hc in range(HC):
                          nc.tensor.matmul(pu, wu[:, hc, ts(fs, P)], h2b[:, hc],
                                           start=(hc == 0), stop=(hc == HC - 1))
                      sg = d2t.tile([P, TT], f32, tag="sg")
                      nc.scalar.activation(out=sg, in_=pg, func=AF.Silu)
                      nc.vector.tensor_tensor(out=inter[:, fg * 4 + fs], in0=sg,
                                              in1=pu, op=OP.mult)
              for hh in range(2):
                  for half in range(2):
                      pdn = [psd.tile([P, TT], f32, tag=f"dn{i}", name=f"pdn{i}")
                             for i in range(2)]
                      for fc in range(FC):
                          wd = d2w.tile([P, 512], bf16, tag="wd", bufs=8)
                          nc.sync.dma_start(out=wd,
                                            in_=r128(wsdT)[:, fc, ts(hh, 512)])
                          for i in range(2):
                              tm = half * 2 + i
                              nc.tensor.matmul(pdn[i], inter[:, fc, ts(tm, P)], wd,
                                               start=(fc == 0), stop=(fc == FC - 1))
                      for i in range(2):
                          tm = half * 2 + i
                          nc.vector.tensor_copy(out=moe[:, tm, ts(hh, 512)],
                                                in_=pdn[i])

              # ---------- routed experts: gather cap=256 tokens each --------
              for ex in range(E):
                  inter_g = d2.tile([P, FC, CAP], fp8, tag="inter_g", bufs=2)
                  h2g = d2.tile([P, HC, CAP], fp8, tag="h2g", bufs=2)
                  y_eb = d2.tile([P, CAP // P, H], bf16, tag="y_eb", bufs=2)
                  SC = SCl[ex]
                  # SCT = transpose(SCw): weighted scatter matrix
                  SCT = d2.tile([P, CAP // P, TT], bf16, tag="SCT", bufs=2)
                  for tc4 in range(4):
                      for gc in range(CAP // P):
                          ptt = psd.tile([P, 2 * P], bf16, tag="pp", bufs=1)
                          ptts = ptt[:, :P]
                          nc.tensor.transpose(ptts, SCwl[ex][:, tc4, ts(gc, P)],
                                              identb)
                          nc.scalar.activation(out=SCT[:, gc, ts(tc4, P)],
                                               in_=ptts, func=AF.Copy)
                  # gather h2 rows: h2g[h, g] = sum_t h2n[t, h] * SC[t, g]
                  # (fp8 x {0,1} sums are exact fp8 values -> plain Copy)
                  for hm in range(HC):
                      pg2 = psd.tile([P, TT], f32, tag="g", bufs=3, name="pg2")[:, :CAP]
                      for tc4 in range(0, 4, 2):
                          nc.tensor.matmul(pg2, h2nb[:, tc4:tc4 + 2, ts(hm, P)],
                                           SC[:, tc4:tc4 + 2],
                                           start=(tc4 == 0), stop=(tc4 == 2),
                                           perf_mode=PM.DoubleRow)
                      nc.scalar.activation(out=h2g[:, hm], in_=pg2, func=AF.Copy)
                  # gate/up on gathered tokens
                  for fg in range(4):
                      wg = d2w.tile([P, HC, 512], fp8, tag="wg")
                      nc.sync.dma_start(out=wg,
                                        in_=r128(wegT[ex])[:, :, ts(fg, 512)])
                      wu = d2w.tile([P, HC, 512], fp8, tag="wu")
                      nc.sync.dma_start(out=wu,
                                        in_=r128(weuT[ex])[:, :, ts(fg, 512)])
                      for fs in range(4):
                          pg = psd.tile([P, TT], f32, tag="g", bufs=3)
                          pgs = pg[:, :CAP]
                          pu = psd.tile([P, TT], f32, tag="u", bufs=2)
                          pus = pu[:, :CAP]
                          for hc in range(0, HC, 2):
                              nc.tensor.matmul(pgs, wg[:, hc:hc + 2, ts(fs, P)],
                                               h2g[:, hc:hc + 2],
                                               start=(hc == 0),
                                               stop=(hc == HC - 2),
                                               perf_mode=PM.DoubleRow)
                          for hc in range(0, HC, 2):
                              nc.tensor.matmul(pus, wu[:, hc:hc + 2, ts(fs, P)],
                                               h2g[:, hc:hc + 2],
                                               start=(hc == 0),
                                               stop=(hc == HC - 2),
                                               perf_mode=PM.DoubleRow)
                          sg = d2t.tile([P, CAP], f32, tag="sgc")
                          nc.scalar.activation(out=sg, in_=pgs, func=AF.Silu,
                                               scale=dq_eg[ex])
                          nc.vector.scalar_tensor_tensor(
                              out=inter_g[:, fg * 4 + fs], in0=pus,
                              scalar=dq_eu_i[ex], in1=sg, op0=OP.mult,
                              op1=OP.mult)
                  # down projection on gathered tokens -> y_eb [gtok, H]
                  for hh in range(2):
                      pdn = [psd.tile([P, TT], f32, tag=f"dn{i}", name=f"pdn{i}")
                             for i in range(CAP // P)]
                      for fc in range(0, FC, 2):
                          wd = d2w.tile([P, 2, 512], fp8, tag="wd", bufs=8)
                          nc.sync.dma_start(out=wd,
                                            in_=r128(wedT[ex])[:, fc:fc + 2,
                                                               ts(hh, 512)])
                          for gm in range(CAP // P):
                              nc.tensor.matmul(pdn[gm][:, :512],
                                               inter_g[:, fc:fc + 2, ts(gm, P)],
                                               wd, start=(fc == 0),
                                               stop=(fc == FC - 2),
                                               perf_mode=PM.DoubleRow)
                      for gm in range(CAP // P):
                          nc.scalar.activation(out=y_eb[:, gm, ts(hh, 512)],
                                               in_=pdn[gm][:, :512],
                                               func=AF.Copy, scale=dq_ed[ex])
                  # scatter-add back: moe[t, h] += sum_g SCT[g, t-block] * y_eb[g, h]
                  for tm in range(TT // P):
                      for hh in range(2):
                          pm = psd.tile([P, TT], f32, tag="dn0", bufs=1, name="pm")
                          pms = pm[:, :512]
                          for gm in range(CAP // P):
                              nc.tensor.matmul(pms, SCT[:, gm, ts(tm, P)],
                                               y_eb[:, gm, ts(hh, 512)],
                                               start=(gm == 0),
                                               stop=(gm == CAP // P - 1))
                          nc.vector.tensor_tensor(out=moe[:, tm, ts(hh, 512)],
                                                  in0=moe[:, tm, ts(hh, 512)],
                                                  in1=pms, op=OP.add)

              for tm in range(TT // P):
                  nc.vector.tensor_tensor(out=xpn[:, tm], in0=xpn[:, tm],
                                          in1=moe[:, tm], op=OP.add)
                  nc.sync.dma_start(out=r128(out)[:, tm], in_=xpn[:, tm])

          d0.release()
          pp.release()
          cst.release()

    _split_multiwaits(nc)
    return nc


# ---------------------------------------------------------------------------
# Host side
# ---------------------------------------------------------------------------

_NC_CACHE = {}


def _get_nc(scales, repeat=1):
    key = f"nc{repeat}-" + ",".join(
        f"{v}" for k in ("eg", "eu", "ed") for v in scales[k])
    if key not in _NC_CACHE:
        _NC_CACHE[key] = build_nc(repeat, scales=scales)
    return _NC_CACHE[key]


def _q8(a, axes=None):
    """Quantize to fp8e4 (max-normal 240) with a single scale; returns
    (quantized array, scale F) with a_q ~= a * F."""
    amax = float(np.abs(a).max())
    F = 224.0 / amax if amax > 0 else 1.0
    return (np.asarray(a, np.float32) * F).astype(ml_dtypes.float8_e4m3), F


def _rope_tables():
    inv_freq = 1.0 / (10000.0 ** (np.arange(0, H, 2, dtype=np.float64) / H))
    t = np.arange(T, dtype=np.float64)
    freqs = np.outer(t, inv_freq)
    emb = np.concatenate([freqs, freqs], axis=-1)          # [T, H]
    return (np.cos(emb).astype(np.float32).T.copy(),
            np.sin(emb).astype(np.float32).T.copy())       # [H, T]


def make_in_maps(inputs):
    bf = ml_dtypes.bfloat16
    x = np.asarray(inputs["x"], np.float32)
    ln1 = np.asarray(inputs["ln1_w"], np.float32)
    ln2 = np.asarray(inputs["ln2_w"], np.float32)

    def tb(a):  # transpose last two dims, contiguous, bf16
        return np.ascontiguousarray(np.swapaxes(a, -1, -2)).astype(bf)

    wkvT = tb(np.asarray(inputs["kv_proj_d"]) * ln1[None, :])
    wqT = tb(np.asarray(inputs["q_proj_d"]) * ln1[None, :])
    wrkT = tb(np.asarray(inputs["rope_k"]) * ln1[None, :])
    wvT = tb(np.asarray(inputs["v_proj_u"]))
    wrqT = tb(np.asarray(inputs["rope_q"]))
    woT = tb(np.asarray(inputs["o_proj"]))
    wrtT = np.ascontiguousarray(
        (np.asarray(inputs["router_w"], np.float32) * ln2[None, :]).T
        .reshape(HC, P, E).transpose(1, 0, 2))
    rbias = np.asarray(inputs["routing_bias"], np.float32).reshape(1, E)

    def tq(a):  # transpose last two dims, contiguous, fp8 + scale
        return _q8(np.ascontiguousarray(np.swapaxes(a, -1, -2)))

    wsgT = tb(np.asarray(inputs["sh_gate"]) * ln2[None, :])
    wsuT = tb(np.asarray(inputs["sh_up"]) * ln2[None, :])
    wsdT = tb(np.asarray(inputs["sh_down"]))
    eg_l = [tq(np.asarray(inputs["ex_gate"][e]) * ln2[None, :]) for e in range(E)]
    eu_l = [tq(np.asarray(inputs["ex_up"][e]) * ln2[None, :]) for e in range(E)]
    ed_l = [tq(np.asarray(inputs["ex_down"][e])) for e in range(E)]
    wegT = np.stack([q for q, _ in eg_l])
    weuT = np.stack([q for q, _ in eu_l])
    wedT = np.stack([q for q, _ in ed_l])
    scales = {"eg": [f for _, f in eg_l], "eu": [f for _, f in eu_l],
              "ed": [f for _, f in ed_l]}

    cosT, sinT = _rope_tables()
    cosb = cosT.astype(bf)
    sinb = sinT.astype(bf)

    xT = np.ascontiguousarray(x.transpose(0, 2, 1))  # [B, H, T]
    iob_np = np.tile(np.arange(256, dtype=np.float32), (P, 1))
    triS_np = np.tril(np.ones((P, P), np.float32), -1).astype(bf)
    ones2d_np = np.ones((P, P), np.float32).astype(bf)

    in_maps = []
    for c in range(N_CORES):
        b, j = c // 4, c % 4
        qoff = 512 * j
        kk = np.arange(TB).reshape(KC, P, 1)
        qq = qoff + np.arange(TT).reshape(1, 1, TT)
        msk = (kk <= qq).astype(bf)
        in_maps.append({
            "xbT": xT[b],
            "xoT": np.ascontiguousarray(xT[b][:, qoff:qoff + TT]),
            "xon": np.ascontiguousarray(x[b][qoff:qoff + TT, :]),
            "cosb": cosb, "sinb": sinb,
            "cosq": np.ascontiguousarray(cosb[:, qoff:qoff + TT]),
            "sinq": np.ascontiguousarray(sinb[:, qoff:qoff + TT]),
            "msk": msk,
            "wkvT": wkvT, "wqT": wqT, "wvT": wvT, "wrqT": wrqT,
            "wrkT": wrkT, "woT": woT, "wrtT": wrtT, "rbias": rbias,
            "wsgT": wsgT, "wsuT": wsuT, "wsdT": wsdT,
            "wegT": wegT, "weuT": weuT, "wedT": wedT,
            "iob": iob_np, "triS": triS_np, "ones2d": ones2d_np,
        })
    return in_maps, scales


def kernel(**inputs):
    in_maps, scales = make_in_maps(inputs)
    import os
    nc = _get_nc(scales)
    trace = bool(int(os.environ.get("KERNEL_TRACE", "0")))
    res = run_bass_kernel_spmd(nc, in_maps, core_ids=list(range(N_CORES)),
                               trace=trace,
                               trace_cores=[0, 3, 7] if trace else None)
    _NC_CACHE["last_result"] = res

    outp = np.empty((B, T, H), np.float32)
    for c in range(N_CORES):
        b, j = c // 4, c % 4
        outp[b, 512 * j:512 * (j + 1), :] = res.results[c]["out"]
    return outp



# revision 45
# speedup vs baseline: 1.5885x; 1.0873x over previous
"""DeepSeek block (MLA attention + shared MLP + 7-expert top-2 MoE) on 8 TRN2
NeuronCores.

Sharding: core c handles batch b=c//4, query block j=c%4 (512 tokens) for
attention/MoE; K/V for the full 2048-token batch slab are computed redundantly
on each of the 4 cores of a batch group (uniform SPMD program, no collectives).
Causality is enforced by 0/1 value masks supplied per core.

Layouts: activations live as [128 partitions = H%128, H//128 chunks, tokens]
("T-layout") so every matmul contraction is on partitions; all weights are
pre-transposed on the host. The MoE down-projection emits [tokens, H] so the
per-token top-2 combine weight is a native per-partition scalar.
"""

import functools

import numpy as np
import ml_dtypes

import concourse.bass as bass
import concourse.tile as tile
from concourse import mybir
from concourse.bass import ds, ts
from concourse.bass_utils import run_bass_kernel_spmd
from concourse.masks import make_identity

f32 = mybir.dt.float32
bf16 = mybir.dt.bfloat16
fp8 = mybir.dt.float8e4
AF = mybir.ActivationFunctionType
OP = mybir.AluOpType
PM = mybir.MatmulPerfMode

# fp8 quantization scales: device activations are stored as value*C_H etc.
C_H = 16.0   # layernorm outputs h/h2 (|x| <~ 5)
C_I = 8.0    # MLP intermediate silu(g)*u, |x| <~ 10
C_L = 32.0   # latent projections kv_lat/q_lat (|x| <~ 4)
C_K = 32.0   # rotary keys (|x| <~ 5)
C_Q = 64.0   # rotary queries (|x| <~ 2)
C_V = 64.0   # values (|x| <~ 2)
LN16 = float(np.log(16.0))  # exp(x+ln16) = 16*exp(x): e_sb kept at ~[8,48]

P = 128
B, T, H, L, F, E = 2, 2048, 1024, 256, 2048, 7
HC, LC, FC = H // P, L // P, F // P  # 8, 2, 16
TT = 512          # own tokens per core
TB = 2048         # batch slab tokens
NTB = TB // 512   # 4 batch token tiles
KC = TB // P      # 16 key chunks
EPS = 1e-5
SCALE = 1.0 / 32.0  # 1/sqrt(H)
N_CORES = 8


def _split_multiwaits(nc, max_waits=1):
    """walrus here supports one sync-wait per instruction; hoist extras onto
    preceding NoOps on the same engine."""
    ctr = 0
    for f in nc.m.functions:
        for bb in f.blocks:
            out = []
            dirty = False
            for inst in bb.instructions:
                si = inst.sync_info
                if si is not None and len(si.on_wait) > max_waits:
                    waits = list(si.on_wait)
                    for w in waits[:-max_waits]:
                        ctr += 1
                        nop = mybir.InstNoOp(name=f"waitnop-{ctr}", ins=[], outs=[])
                        nop.engine = inst.engine
                        nop.sync_info = mybir.SyncInfo(on_wait=[w], on_update=[])
                        out.append(nop)
                    inst.sync_info = mybir.SyncInfo(
                        on_wait=waits[-max_waits:], on_update=list(si.on_update)
                    )
                    dirty = True
                out.append(inst)
            if dirty:
                bb.instructions = out
    return ctr


def _bn_mean_rstd(nc, pool, src_ap, epsb):
    """src_ap [128, 1024] f32 -> mr [128, 2] (mean, rstd) via bn_stats."""
    stats = pool.tile([P, 2, 6], f32, tag="bn_stats")
    nc.vector.bn_stats(out=stats[:, 0], in_=src_ap[:, 0:512])
    nc.vector.bn_stats(out=stats[:, 1], in_=src_ap[:, 512:1024])
    mv = pool.tile([P, 2], f32, tag="bn_mv")
    nc.vector.bn_aggr(out=mv, in_=stats)
    mr = pool.tile([P, 2], f32, tag="bn_mr")
    nc.vector.tensor_copy(out=mr[:, 0:1], in_=mv[:, 0:1])
    nc.scalar.activation(out=mr[:, 1:2], in_=mv[:, 1:2], func=AF.Sqrt, bias=epsb)
    nc.vector.reciprocal(out=mr[:, 1:2], in_=mr[:, 1:2])
    return mr


def build_nc(repeat=1, scales=None):
    # scales: host-side fp8 weight quant factors (w_q = w * F); dequant is
    # folded into the activation/copy ops after each PSUM group.
    sc = scales or {}
    dq_eg = [1.0 / (f * C_H) for f in sc["eg"]]
    dq_eu_i = [C_I / (f * C_H) for f in sc["eu"]]
    dq_ed = [1.0 / (f * C_I) for f in sc["ed"]]
    dq_qlat = C_L / (sc["q"] * C_H)
    dq_qrf = 1.0 / (sc["rq"] * C_L)
    dq_kvl = C_L / (sc["kv"] * C_H)
    dq_v = C_V / (sc["v"] * C_L)
    dq_krf = 1.0 / (sc["rk"] * C_H)
    dq_o = 1.0 / (sc["o"] * C_V)
    exp_scale = SCALE / (C_K * C_Q)

    nc = bass.Bass()

    def din(name, shape, dt=bf16):
        return nc.declare_dram_parameter(name, list(shape), dt, isOutput=False)

    xbT = din("xbT", [H, TB])
    xoT = din("xoT", [H, TT])
    xon = din("xon", [TT, H], f32)
    cosb = din("cosb", [H, TB])
    sinb = din("sinb", [H, TB])
    cosq = din("cosq", [H, TT])
    sinq = din("sinq", [H, TT])
    msk = din("msk", [KC, P, TT], fp8)
    wkvT = din("wkvT", [H, L], fp8)
    wqT = din("wqT", [H, L], fp8)
    wvT = din("wvT", [L, H], fp8)
    wrqT = din("wrqT", [L, H], fp8)
    wrkT = din("wrkT", [H, H], fp8)
    woT = din("woT", [H, H], fp8)
    wrtT = din("wrtT", [P, HC, E], f32)
    rbias = din("rbias", [1, E], f32)
    wsgT = din("wsgT", [H, F])
    wsuT = din("wsuT", [H, F])
    wsdT = din("wsdT", [F, H])
    iob = din("iob", [P, 256], f32)
    triS = din("triS", [P, P])
    ones2d = din("ones2d", [P, P])
    wegT = din("wegT", [E, H, F], fp8)
    weuT = din("weuT", [E, H, F], fp8)
    wedT = din("wedT", [E, F, H], fp8)
    out = nc.declare_dram_parameter("out", [TT, H], f32, isOutput=True)

    r128 = lambda ap: ap.rearrange("(c p) x -> p c x", p=P)

    with tile.TileContext(nc) as tc:
      for rep in range(repeat):
          cst = tc.alloc_tile_pool(name=f"cst{rep}", bufs=1)
          pp = tc.alloc_tile_pool(name=f"pp{rep}", bufs=1)       # persist: qrope, yn, xpn
          psg = tc.alloc_tile_pool(name=f"psg{rep}", bufs=4, space="PSUM")

          ones128b = cst.tile([P, 1], bf16)
          nc.vector.memset(ones128b, 1.0)
          ones128q = cst.tile([P, 1], fp8)
          nc.vector.memset(ones128q, 1.0)
          ones1b = cst.tile([1, P], bf16)
          nc.vector.memset(ones1b, 1.0)
          ones1bC = cst.tile([1, P], bf16)
          nc.vector.memset(ones1bC, C_H)
          ones1f = cst.tile([1, P], f32)
          nc.vector.memset(ones1f, 1.0)
          ones1fC = cst.tile([1, P], f32)
          nc.vector.memset(ones1fC, C_H)
          epsb1 = cst.tile([1, 1], f32)
          nc.vector.memset(epsb1, EPS)
          epsb128 = cst.tile([P, 1], f32)
          nc.vector.memset(epsb128, EPS)
          ln16b = cst.tile([P, 1], f32)
          nc.vector.memset(ln16b, LN16)
          ident = cst.tile([P, P], f32)
          make_identity(nc, ident)
          wrt_sb = cst.tile([P, HC, E], f32)
          nc.sync.dma_start(out=wrt_sb, in_=wrtT[:, :, :])
          rbias_sb = cst.tile([1, E], f32)
          nc.sync.dma_start(out=rbias_sb, in_=rbias[:, :])
          iob_sb = cst.tile([P, 256], f32)
          nc.sync.dma_start(out=iob_sb, in_=iob[:, :])
          triS_sb = cst.tile([P, P], bf16)
          nc.sync.dma_start(out=triS_sb, in_=triS[:, :])
          ones2d_sb = cst.tile([P, P], bf16)
          nc.sync.dma_start(out=ones2d_sb, in_=ones2d[:, :])

          qrope = pp.tile([P, HC, TT], fp8)
          yn = pp.tile([P, HC, TT], fp8)
          xpn = pp.tile([P, TT // P, H], f32)

          bv = tc.alloc_tile_pool(name=f"bv{rep}", bufs=1)
          v_sb = bv.tile([P, KC, H], fp8)
          krope = bv.tile([P, HC, TB], fp8)

          bw = tc.alloc_tile_pool(name=f"bw{rep}", bufs=1)
          wkv_sb = bw.tile([P, HC, L], fp8)
          nc.sync.dma_start(out=wkv_sb, in_=r128(wkvT))
          wq_sb = bw.tile([P, HC, L], fp8)
          nc.sync.dma_start(out=wq_sb, in_=r128(wqT))
          wv_sb = bw.tile([P, LC, H], fp8)
          nc.sync.dma_start(out=wv_sb, in_=r128(wvT))
          wrq_sb = bw.tile([P, LC, H], fp8)
          nc.sync.dma_start(out=wrq_sb, in_=r128(wrqT))
          wrk_sb = bw.tile([P, HC, H], fp8)
          nc.sync.dma_start(out=wrk_sb, in_=r128(wrkT))

          # ================== OWN pipeline: h_own -> q_lat -> qrope =========
          with tc.tile_pool(name=f"own{rep}", bufs=1) as own, \
               tc.tile_pool(name=f"ownt{rep}", bufs=2) as ownt:
              muT = own.tile([1, TT], f32)
              rsT = own.tile([1, TT], f32)
              for tm in range(TT // P):
                  xon_t = ownt.tile([P, H], f32, tag="xon_t")
                  nc.sync.dma_start(out=xon_t, in_=r128(xon)[:, tm])
                  mr = _bn_mean_rstd(nc, ownt, xon_t, epsb128)
                  ptm = psg.tile([1, P], f32, tag="pb1")
                  nc.tensor.transpose(ptm, mr[:, 0:1], ident)
                  nc.vector.tensor_copy(out=muT[:, ts(tm, P)], in_=ptm[0:1, :])
                  ptr = psg.tile([1, P], f32, tag="pb1")
                  nc.tensor.transpose(ptr, mr[:, 1:2], ident)
                  nc.vector.tensor_copy(out=rsT[:, ts(tm, P)], in_=ptr[0:1, :])
              pmu = psg.tile([P, TT], f32, tag="pb1")
              nc.tensor.matmul(pmu, ones1f, muT, start=True, stop=True)
              muB = own.tile([P, TT], f32)
              nc.vector.tensor_copy(out=muB, in_=pmu)
              prs = psg.tile([P, TT], f32, tag="pb1")
              nc.tensor.matmul(prs, ones1fC, rsT, start=True, stop=True)
              rsB = own.tile([P, TT], f32)  # C_H * rstd
              nc.vector.tensor_copy(out=rsB, in_=prs)

              h_own = own.tile([P, HC, TT], fp8)  # C_H * h
              for hc in range(HC):
                  xoT_t = ownt.tile([P, TT], bf16, tag="xoT_t")
                  nc.sync.dma_start(out=xoT_t, in_=r128(xoT)[:, hc])
                  tmp = ownt.tile([P, TT], f32, tag="ot_f32")
                  nc.vector.tensor_tensor(out=tmp, in0=xoT_t, in1=muB,
                                          op=OP.subtract)
                  nc.vector.tensor_tensor(out=h_own[:, hc], in0=tmp, in1=rsB,
                                          op=OP.mult)
              qlat = own.tile([P, LC, TT], fp8)  # C_L * q_lat
              for lc in range(LC):
                  pq = psg.tile([P, TT], f32, tag="pb1")
                  for hc in range(0, HC, 2):
                      nc.tensor.matmul(pq, wq_sb[:, hc:hc + 2, ts(lc, P)],
                                       h_own[:, hc:hc + 2],
                                       start=(hc == 0), stop=(hc == HC - 2),
                                       perf_mode=PM.DoubleRow)
                  nc.scalar.activation(out=qlat[:, lc], in_=pq, func=AF.Copy,
                                       scale=dq_qlat)
              qrf = own.tile([P, HC, TT], bf16)
              for hm in range(HC):
                  pq = psg.tile([P, TT], f32, tag="pb1")
                  nc.tensor.matmul(pq, wrq_sb[:, 0:2, ts(hm, P)], qlat[:, 0:2],
                                   start=True, stop=True,
                                   perf_mode=PM.DoubleRow)
                  nc.scalar.activation(out=qrf[:, hm], in_=pq, func=AF.Copy,
                                       scale=dq_qrf)
              cosq_sb = own.tile([P, HC, TT], bf16)  # host-prescaled by C_Q
              nc.sync.dma_start(out=cosq_sb, in_=r128(cosq))
              sinq_sb = own.tile([P, HC, TT], bf16)
              nc.sync.dma_start(out=sinq_sb, in_=r128(sinq))
              for c in range(HC):
                  t1 = ownt.tile([P, TT], bf16, tag="ot_b1")
                  t2 = ownt.tile([P, TT], bf16, tag="ot_b2")
                  nc.vector.tensor_tensor(out=t1, in0=qrf[:, c], in1=cosq_sb[:, c],
                                          op=OP.mult)
                  nc.vector.tensor_tensor(out=t2, in0=qrf[:, (c + 4) % 8],
                                          in1=sinq_sb[:, c], op=OP.mult)
                  nc.vector.tensor_tensor(out=qrope[:, c], in0=t1, in1=t2,
                                          op=OP.subtract if c < 4 else OP.add)

          # ================== BATCH loop: h, kv_lat, v, krope ===============
          with tc.tile_pool(name=f"bst{rep}", bufs=1, space="PSUM") as bst, \
               tc.tile_pool(name=f"bt{rep}", bufs=1) as bt, \
               tc.tile_pool(name=f"btt{rep}", bufs=2) as btt:
              for t in range(NTB):
                  tsl = ts(t, 512)
                  xbb = bt.tile([P, HC, 512], bf16, tag="xbb", bufs=2)
                  psA = bst.tile([1, 512], f32, tag="psA")
                  psB = bst.tile([1, 512], f32, tag="psB")
                  for hc in range(HC):
                      nc.sync.dma_start(out=xbb[:, hc], in_=r128(xbT)[:, hc, tsl])
                      xsq = btt.tile([P, 512], bf16, tag="xsq", bufs=1)
                      nc.vector.tensor_tensor(out=xsq, in0=xbb[:, hc],
                                              in1=xbb[:, hc], op=OP.mult)
                      nc.tensor.matmul(psA, ones128b, xbb[:, hc],
                                       start=(hc == 0), stop=(hc == HC - 1))
                      nc.tensor.matmul(psB, ones128b, xsq,
                                       start=(hc == 0), stop=(hc == HC - 1))
                  mu = btt.tile([1, 512], f32, tag="mu", bufs=1)
                  nc.vector.tensor_scalar_mul(out=mu, in0=psA, scalar1=1.0 / H)
                  mu2 = btt.tile([1, 512], f32, tag="mu2", bufs=1)
                  nc.vector.tensor_tensor(out=mu2, in0=mu, in1=mu, op=OP.mult)
                  var = btt.tile([1, 512], f32, tag="var", bufs=1)
                  nc.vector.scalar_tensor_tensor(out=var, in0=psB, scalar=1.0 / H,
                                                 in1=mu2, op0=OP.mult, op1=OP.subtract)
                  nc.scalar.activation(out=var, in_=var, func=AF.Sqrt, bias=epsb1)
                  rs = var
                  nc.vector.reciprocal(out=rs, in_=rs)
                  mub = btt.tile([1, 512], bf16, tag="mub", bufs=1)
                  nc.vector.tensor_copy(out=mub, in_=mu)
                  rsb = btt.tile([1, 512], bf16, tag="rsb", bufs=1)
                  nc.vector.tensor_copy(out=rsb, in_=rs)
                  pmu = psg.tile([P, 512], f32, tag="pb1")
                  nc.tensor.matmul(pmu, ones1b, mub, start=True, stop=True)
                  muB = btt.tile([P, 512], bf16, tag="muB", bufs=1)
                  nc.vector.tensor_copy(out=muB, in_=pmu)
                  prs = psg.tile([P, 512], f32, tag="pb1")
                  nc.tensor.matmul(prs, ones1bC, rsb, start=True, stop=True)
                  rsB = btt.tile([P, 512], bf16, tag="rsB", bufs=1)  # C_H*rstd
                  nc.vector.tensor_copy(out=rsB, in_=prs)

                  h_t = bt.tile([P, HC, 512], fp8, tag="h_t", bufs=2)  # C_H*h
                  for hc in range(HC):
                      tmp = btt.tile([P, 512], bf16, tag="htmp", bufs=1)
                      nc.vector.tensor_tensor(out=tmp, in0=xbb[:, hc], in1=muB,
                                              op=OP.subtract)
                      nc.vector.tensor_tensor(out=h_t[:, hc], in0=tmp, in1=rsB,
                                              op=OP.mult)
                  kvl = bt.tile([P, LC, 512], fp8, tag="kvl")  # C_L*kv_lat
                  for lc in range(LC):
                      pk = psg.tile([P, 512], f32, tag="pb1")
                      for hc in range(0, HC, 2):
                          nc.tensor.matmul(pk, wkv_sb[:, hc:hc + 2, ts(lc, P)],
                                           h_t[:, hc:hc + 2],
                                           start=(hc == 0), stop=(hc == HC - 2),
                                           perf_mode=PM.DoubleRow)
                      nc.scalar.activation(out=kvl[:, lc], in_=pk, func=AF.Copy,
                                           scale=dq_kvl)
                  for i in range(4):
                      for hh in range(2):
                          pv = psg.tile([P, 512], f32, tag="pb1")
                          nc.tensor.matmul(pv, kvl[:, 0:2, ts(i, P)],
                                           wv_sb[:, 0:2, ts(hh, 512)],
                                           start=True, stop=True,
                                           perf_mode=PM.DoubleRow)
                          nc.scalar.activation(out=v_sb[:, t * 4 + i, ts(hh, 512)],
                                               in_=pv, func=AF.Copy, scale=dq_v)
                  krf = bt.tile([P, HC, 512], bf16, tag="krf")
                  for hm in range(HC):
                      pk = psg.tile([P, 512], f32, tag="pb1")
                      for hc in range(0, HC, 2):
                          nc.tensor.matmul(pk, wrk_sb[:, hc:hc + 2, ts(hm, P)],
                                           h_t[:, hc:hc + 2],
                                           start=(hc == 0), stop=(hc == HC - 2),
                                           perf_mode=PM.DoubleRow)
                      nc.scalar.activation(out=krf[:, hm], in_=pk, func=AF.Copy,
                                           scale=dq_krf)
                  for c in range(HC):
                      cosc = btt.tile([P, 512], bf16, tag="cosc")
                      nc.sync.dma_start(out=cosc, in_=r128(cosb)[:, c, tsl])
                      sinc = btt.tile([P, 512], bf16, tag="sinc")
                      nc.sync.dma_start(out=sinc, in_=r128(sinb)[:, c, tsl])
                      t1 = btt.tile([P, 512], bf16, tag="kt1")
                      t2 = btt.tile([P, 512], bf16, tag="kt2")
                      nc.vector.tensor_tensor(out=t1, in0=krf[:, c], in1=cosc,
                                              op=OP.mult)
                      nc.vector.tensor_tensor(out=t2, in0=krf[:, (c + 4) % 8],
                                              in1=sinc, op=OP.mult)
                      nc.vector.tensor_tensor(out=krope[:, c, tsl], in0=t1, in1=t2,
                                              op=OP.subtract if c < 4 else OP.add)

          # ================== Attention ====================================
          with tc.tile_pool(name=f"cp{rep}", bufs=1) as cp, \
               tc.tile_pool(name=f"cpt{rep}", bufs=2) as cpt:
              mask_sb = cp.tile([P, KC, TT], fp8)
              nc.sync.dma_start(out=mask_sb, in_=msk.rearrange("k p q -> p k q"))
              e_sb = cp.tile([P, KC, TT], fp8)
              for kc in range(KC):
                  ps = psg.tile([P, TT], f32, tag="pb1")
                  for hc in range(0, HC, 2):
                      nc.tensor.matmul(ps, krope[:, hc:hc + 2, ds(kc * P, P)],
                                       qrope[:, hc:hc + 2],
                                       start=(hc == 0), stop=(hc == HC - 2),
                                       perf_mode=PM.DoubleRow)
                  # e = 16*exp(score): bias ln16 keeps fp8 resolution
                  nc.scalar.activation(out=e_sb[:, kc], in_=ps, func=AF.Exp,
                                       scale=exp_scale, bias=ln16b)
                  nc.vector.tensor_tensor(out=e_sb[:, kc], in0=e_sb[:, kc],
                                          in1=mask_sb[:, kc], op=OP.mult)
              pd = psg.tile([1, TT], f32, tag="pb1")
              for kc in range(KC):
                  nc.tensor.matmul(pd, ones128q, e_sb[:, kc],
                                   start=(kc == 0), stop=(kc == KC - 1))
              rden = cp.tile([1, TT], f32)
              nc.vector.reciprocal(out=rden, in_=pd)
              prb = psg.tile([P, TT], f32, tag="pb1")
              nc.tensor.matmul(prb, ones1f, rden, start=True, stop=True)
              rdenB = cp.tile([P, TT], f32)
              nc.vector.tensor_copy(out=rdenB, in_=prb)
              for hm in range(HC):
                  py = psg.tile([P, TT], f32, tag="pb1")
                  for kc in range(0, KC, 2):
                      nc.tensor.matmul(py, v_sb[:, kc:kc + 2, ds(hm * P, P)],
                                       e_sb[:, kc:kc + 2],
                                       start=(kc == 0), stop=(kc == KC - 2),
                                       perf_mode=PM.DoubleRow)
                  # py*rden = C_V * y  (the 16*exp factors cancel)
                  nc.vector.tensor_tensor(out=yn[:, hm], in0=py, in1=rdenB,
                                          op=OP.mult)

          bw.release()   # frees attention weights
          # (bv still open: v/krope dead but space reclaimed below after cp closed)
          bv.release()

          # ================== o_proj + LN2 + router ========================
          d0 = tc.alloc_tile_pool(name=f"d0{rep}", bufs=1)
          h2b = d0.tile([P, HC, TT], bf16)
          h2fT = d0.tile([P, HC, TT], f32)
          probs = d0.tile([P, TT // P, 8], f32)
          nc.vector.memset(probs, -1e30)
          cmb = d0.tile([P, TT // P, 8], f32)
          nc.vector.memset(cmb, 0.0)
          h2nb = d0.tile([P, TT // P, H], fp8)

          with tc.tile_pool(name=f"d1{rep}", bufs=1) as d1, \
               tc.tile_pool(name=f"d1t{rep}", bufs=2) as d1t:
              wo_sb = d1.tile([P, HC, H], fp8)
              nc.sync.dma_start(out=wo_sb, in_=r128(woT))
              xon_sb = d1.tile([P, TT // P, H], f32)
              nc.sync.dma_start(out=xon_sb, in_=r128(xon))
              h2n32 = d1.tile([P, TT // P, H], f32)
              for tm in range(TT // P):
                  for hh in range(2):
                      po = psg.tile([P, 512], f32, tag="pb1")
                      for hc in range(0, HC, 2):
                          nc.tensor.matmul(po, yn[:, hc:hc + 2, ts(tm, P)],
                                           wo_sb[:, hc:hc + 2, ts(hh, 512)],
                                           start=(hc == 0), stop=(hc == HC - 2),
                                           perf_mode=PM.DoubleRow)
                      nc.vector.scalar_tensor_tensor(
                          out=xpn[:, tm, ts(hh, 512)], in0=po, scalar=dq_o,
                          in1=xon_sb[:, tm, ts(hh, 512)], op0=OP.mult,
                          op1=OP.add)
              for tm in range(TT // P):
                  mr = _bn_mean_rstd(nc, d1t, xpn[:, tm], epsb128)
                  mrc = d1t.tile([P, 1], f32, tag="mrc")
                  nc.vector.tensor_scalar_mul(out=mrc, in0=mr[:, 1:2],
                                              scalar1=C_H)
                  nc.vector.tensor_scalar(out=h2nb[:, tm], in0=xpn[:, tm],
                                          scalar1=mr[:, 0:1], scalar2=mrc,
                                          op0=OP.subtract, op1=OP.mult)
                  nc.vector.tensor_scalar(out=h2n32[:, tm], in0=xpn[:, tm],
                                          scalar1=mr[:, 0:1], scalar2=mr[:, 1:2],
                                          op0=OP.subtract, op1=OP.mult)
              # h2fT/h2b [h, t] via f32 PE transposes of h2n32 (f32 keeps the
              # router's top-2 selection faithful to the reference -- bf16
              # logits flip near-ties, and each flip swaps a whole token's
              # expert output)
              for tm in range(TT // P):
                  for hc in range(HC):
                      ptf = psg.tile([P, TT], f32, tag="pb1")
                      ptfs = ptf[:, :P]
                      nc.tensor.transpose(ptfs, h2n32[:, tm, ts(hc, P)], ident)
                      nc.scalar.activation(out=h2fT[:, hc, ts(tm, P)],
                                           in_=ptfs, func=AF.Copy)
                      nc.vector.tensor_copy(out=h2b[:, hc, ts(tm, P)],
                                            in_=ptfs)

              # router: fp32 matmuls, tokens on partitions
              for tm in range(TT // P):
                  pr = psg.tile([P, TT], f32, tag="pb1")
                  prr = pr[:, :E]
                  for hc in range(HC):
                      nc.tensor.matmul(prr, h2fT[:, hc, ts(tm, P)], wrt_sb[:, hc],
                                       start=(hc == 0), stop=False)
                  nc.tensor.matmul(prr, ones1f, rbias_sb, start=False, stop=True)
                  nc.scalar.activation(out=probs[:, tm, :E], in_=prr, func=AF.Sigmoid)
                  top8 = d1t.tile([P, 8], f32, tag="top8")
                  nc.vector.max(out=top8, in_=probs[:, tm])
                  nc.vector.tensor_scalar(out=cmb[:, tm, :E], in0=probs[:, tm, :E],
                                          scalar1=top8[:, 1:2], scalar2=None,
                                          op0=OP.is_ge)
                  nc.vector.tensor_tensor(out=cmb[:, tm, :E], in0=cmb[:, tm, :E],
                                          in1=probs[:, tm, :E], op=OP.mult)

          psg.release()

          # ================== MoE: shared + 7 experts (gathered) ===========
          CAP = 256
          with tc.tile_pool(name=f"d2{rep}", bufs=1) as d2, \
               tc.tile_pool(name=f"d2w{rep}", bufs=2) as d2w, \
               tc.tile_pool(name=f"d2t{rep}", bufs=2) as d2t, \
               tc.tile_pool(name=f"psd{rep}", bufs=1, space="PSUM") as psd:
              identb = d2.tile([P, P], bf16)
              make_identity(nc, identb)
              moe = d2.tile([P, TT // P, H], f32)
              inter = d2.tile([P, FC, TT], bf16)

              # ---------- routed-expert gather matrices, hoisted ------------
              # Selection + prefix-sum batched across all 7 experts, then
              # per-expert one-hot SC/SCw on gpsimd -- all issued before the
              # shared expert so gpsimd runs ahead of the routed matmul
              # stream (was ~12us of tensor idle per expert).
              SCl = [d2.tile([P, TT // P, CAP], fp8, tag=f"SCl{e}",
                             name=f"SCl{e}") for e in range(E)]
              SCwl = [d2.tile([P, TT // P, CAP], bf16, tag=f"SCwl{e}",
                              name=f"SCwl{e}") for e in range(E)]
              selb = d2t.tile([P, 4, E], bf16, tag="selb")
              nc.gpsimd.tensor_scalar(out=selb, in0=cmb[:, :, :E],
                                      scalar1=0.0, scalar2=None, op0=OP.is_gt)
              self32 = d2t.tile([P, 4, E], f32, tag="self32")
              nc.gpsimd.tensor_scalar(out=self32, in0=cmb[:, :, :E],
                                      scalar1=0.0, scalar2=None, op0=OP.is_gt)
              ppos = psd.tile([P, 2 * P], f32, tag="pp", bufs=1,
                              name="ppos").rearrange("p (a b) -> p a b",
                                                     a=4)[:, :, :E]
              for tc4 in range(4):
                  for tcp in range(tc4 + 1):
                      blk = triS_sb if tcp == tc4 else ones2d_sb
                      nc.tensor.matmul(ppos[:, tc4], blk, selb[:, tcp],
                                       start=(tcp == 0), stop=(tcp == tc4))
              posx = d2t.tile([P, 4, E], f32, tag="posx")
              nc.vector.tensor_scalar(out=posx, in0=ppos, scalar1=1.0,
                                      scalar2=None, op0=OP.add)
              nc.gpsimd.tensor_tensor(out=posx, in0=posx, in1=self32,
                                      op=OP.mult)
              nc.gpsimd.tensor_scalar(out=posx, in0=posx, scalar1=-1.0,
                                      scalar2=None, op0=OP.add)
              for ex in range(E):
                  for tc4 in range(4):
                      nc.gpsimd.tensor_scalar(out=SCl[ex][:, tc4], in0=iob_sb,
                                              scalar1=posx[:, tc4, ex:ex + 1],
                                              scalar2=None, op0=OP.is_equal)
                      nc.gpsimd.tensor_scalar(out=SCwl[ex][:, tc4],
                                              in0=iob_sb,
                                              scalar1=posx[:, tc4, ex:ex + 1],
                                              scalar2=cmb[:, tc4, ex:ex + 1],
                                              op0=OP.is_equal, op1=OP.mult)

              # ---------- shared expert: dense over all 512 own tokens ------
              for fg in range(4):
                  wg = d2w.tile([P, HC, 512], bf16, tag="wg")
                  nc.sync.dma_start(out=wg, in_=r128(wsgT)[:, :, ts(fg, 512)])
                  wu = d2w.tile([P, HC, 512], bf16, tag="wu")
                  nc.sync.dma_start(out=wu, in_=r128(wsuT)[:, :, ts(fg, 512)])
                  for fs in range(4):
                      pg = psd.tile([P, TT], f32, tag="g", bufs=3)
                      pu = psd.tile([P, TT], f32, tag="u", bufs=2)
                      for hc in range(HC):
                          nc.tensor.matmul(pg, wg[:, hc, ts(fs, P)], h2b[:, hc],
                                           start=(hc == 0), stop=(hc == HC - 1))
                      for hc in range(HC):
                          nc.tensor.matmul(pu, wu[:, hc, ts(fs, P)], h2b[:, hc],
                                           start=(hc == 0), stop=(hc == HC - 1))
                      sg = d2t.tile([P, TT], f32, tag="sg")
                      nc.scalar.activation(out=sg, in_=pg, func=AF.Silu)
                      nc.vector.tensor_tensor(out=inter[:, fg * 4 + fs], in0=sg,
                                              in1=pu, op=OP.mult)
              for hh in range(2):
                  for half in range(2):
                      pdn = [psd.tile([P, TT], f32, tag=f"dn{i}", name=f"pdn{i}")
                             for i in range(2)]
                      for fc in range(FC):
                          wd = d2w.tile([P, 512], bf16, tag="wd", bufs=8)
                          nc.sync.dma_start(out=wd,
                                            in_=r128(wsdT)[:, fc, ts(hh, 512)])
                          for i in range(2):
                              tm = half * 2 + i
                              nc.tensor.matmul(pdn[i], inter[:, fc, ts(tm, P)], wd,
                                               start=(fc == 0), stop=(fc == FC - 1))
                      for i in range(2):
                          tm = half * 2 + i
                          nc.vector.tensor_copy(out=moe[:, tm, ts(hh, 512)],
                                                in_=pdn[i])

              # ---------- routed experts: gather cap=256 tokens each --------
              for ex in range(E):
                  inter_g = d2.tile([P, FC, CAP], fp8, tag="inter_g", bufs=2)
                  h2g = d2.tile([P, HC, CAP], fp8, tag="h2g", bufs=2)
                  y_eb = d2.tile([P, CAP // P, H], bf16, tag="y_eb", bufs=2)
                  SC = SCl[ex]
                  # SCT = transpose(SCw): weighted scatter matrix
                  SCT = d2.tile([P, CAP // P, TT], bf16, tag="SCT", bufs=2)
                  for tc4 in range(4):
                      for gc in range(CAP // P):
                          ptt = psd.tile([P, 2 * P], bf16, tag="pp", bufs=1)
                          ptts = ptt[:, :P]
                          nc.tensor.transpose(ptts, SCwl[ex][:, tc4, ts(gc, P)],
                                              identb)
                          nc.scalar.activation(out=SCT[:, gc, ts(tc4, P)],
                                               in_=ptts, func=AF.Copy)
                  # gather h2 rows: h2g[h, g] = sum_t h2n[t, h] * SC[t, g]
                  # (fp8 x {0,1} sums are exact fp8 values -> plain Copy)
                  for hm in range(HC):
                      pg2 = psd.tile([P, TT], f32, tag="g", bufs=3, name="pg2")[:, :CAP]
                      for tc4 in range(0, 4, 2):
                          nc.tensor.matmul(pg2, h2nb[:, tc4:tc4 + 2, ts(hm, P)],
                                           SC[:, tc4:tc4 + 2],
                                           start=(tc4 == 0), stop=(tc4 == 2),
                                           perf_mode=PM.DoubleRow)
                      nc.scalar.activation(out=h2g[:, hm], in_=pg2, func=AF.Copy)
                  # gate/up on gathered tokens
                  for fg in range(4):
                      wg = d2w.tile([P, HC, 512], fp8, tag="wg")
                      nc.sync.dma_start(out=wg,
                                        in_=r128(wegT[ex])[:, :, ts(fg, 512)])
                      wu = d2w.tile([P, HC, 512], fp8, tag="wu")
                      nc.sync.dma_start(out=wu,
                                        in_=r128(weuT[ex])[:, :, ts(fg, 512)])
                      for fs in range(4):
                          pg = psd.tile([P, TT], f32, tag="g", bufs=3)
                          pgs = pg[:, :CAP]
                          pu = psd.tile([P, TT], f32, tag="u", bufs=2)
                          pus = pu[:, :CAP]
                          for hc in range(0, HC, 2):
                              nc.tensor.matmul(pgs, wg[:, hc:hc + 2, ts(fs, P)],
                                               h2g[:, hc:hc + 2],
                                               start=(hc == 0),
                                               stop=(hc == HC - 2),
                                               perf_mode=PM.DoubleRow)
                          for hc in range(0, HC, 2):
                              nc.tensor.matmul(pus, wu[:, hc:hc + 2, ts(fs, P)],
                                               h2g[:, hc:hc + 2],
                                               start=(hc == 0),
                                               stop=(hc == HC - 2),
                                               perf_mode=PM.DoubleRow)
                          sg = d2t.tile([P, CAP], f32, tag="sgc")
                          nc.scalar.activation(out=sg, in_=pgs, func=AF.Silu,
                                               scale=dq_eg[ex])
                          nc.vector.scalar_tensor_tensor(
                              out=inter_g[:, fg * 4 + fs], in0=pus,
                              scalar=dq_eu_i[ex], in1=sg, op0=OP.mult,
                              op1=OP.mult)
                  # down projection on gathered tokens -> y_eb [gtok, H]
                  for hh in range(2):
                      pdn = [psd.tile([P, TT], f32, tag=f"dn{i}", name=f"pdn{i}")
                             for i in range(CAP // P)]
                      for fc in range(0, FC, 2):
                          wd = d2w.tile([P, 2, 512], fp8, tag="wd", bufs=8)
                          nc.sync.dma_start(out=wd,
                                            in_=r128(wedT[ex])[:, fc:fc + 2,
                                                               ts(hh, 512)])
                          for gm in range(CAP // P):
                              nc.tensor.matmul(pdn[gm][:, :512],
                                               inter_g[:, fc:fc + 2, ts(gm, P)],
                                               wd, start=(fc == 0),
                                               stop=(fc == FC - 2),
                                               perf_mode=PM.DoubleRow)
                      for gm in range(CAP // P):
                          nc.scalar.activation(out=y_eb[:, gm, ts(hh, 512)],
                                               in_=pdn[gm][:, :512],
                                               func=AF.Copy, scale=dq_ed[ex])
                  # scatter-add back: moe[t, h] += sum_g SCT[g, t-block] * y_eb[g, h]
                  for tm in range(TT // P):
                      for hh in range(2):
                          pm = psd.tile([P, TT], f32, tag="dn0", bufs=1, name="pm")
                          pms = pm[:, :512]
                          for gm in range(CAP // P):
                              nc.tensor.matmul(pms, SCT[:, gm, ts(tm, P)],
                                               y_eb[:, gm, ts(hh, 512)],
                                               start=(gm == 0),
                                               stop=(gm == CAP // P - 1))
                          nc.vector.tensor_tensor(out=moe[:, tm, ts(hh, 512)],
                                                  in0=moe[:, tm, ts(hh, 512)],
                                                  in1=pms, op=OP.add)

              for tm in range(TT // P):
                  nc.vector.tensor_tensor(out=xpn[:, tm], in0=xpn[:, tm],
                                          in1=moe[:, tm], op=OP.add)
                  nc.sync.dma_start(out=r128(out)[:, tm], in_=xpn[:, tm])

          d0.release()
          pp.release()
          cst.release()

    _split_multiwaits(nc)
    return nc


# ---------------------------------------------------------------------------
# Host side
# ---------------------------------------------------------------------------

_NC_CACHE = {}


def _get_nc(scales, repeat=1):
    key = f"nc{repeat}-" + ",".join(
        f"{v}" for k in ("eg", "eu", "ed") for v in scales[k])
    if key not in _NC_CACHE:
        _NC_CACHE[key] = build_nc(repeat, scales=scales)
    return _NC_CACHE[key]


def _q8(a, axes=None):
    """Quantize to fp8e4 (max-normal 240) with a single scale; returns
    (quantized array, scale F) with a_q ~= a * F."""
    amax = float(np.abs(a).max())
    F = 224.0 / amax if amax > 0 else 1.0
    return (np.asarray(a, np.float32) * F).astype(ml_dtypes.float8_e4m3), F


def _rope_tables():
    inv_freq = 1.0 / (10000.0 ** (np.arange(0, H, 2, dtype=np.float64) / H))
    t = np.arange(T, dtype=np.float64)
    freqs = np.outer(t, inv_freq)
    emb = np.concatenate([freqs, freqs], axis=-1)          # [T, H]
    return (np.cos(emb).astype(np.float32).T.copy(),
            np.sin(emb).astype(np.float32).T.copy())       # [H, T]


def make_in_maps(inputs):
    bf = ml_dtypes.bfloat16
    x = np.asarray(inputs["x"], np.float32)
    ln1 = np.asarray(inputs["ln1_w"], np.float32)
    ln2 = np.asarray(inputs["ln2_w"], np.float32)

    def tb(a):  # transpose last two dims, contiguous, bf16
        return np.ascontiguousarray(np.swapaxes(a, -1, -2)).astype(bf)

    def tq(a):  # transpose last two dims, contiguous, fp8 + scale
        return _q8(np.ascontiguousarray(np.swapaxes(a, -1, -2)))

    wkvT, f_kv = tq(np.asarray(inputs["kv_proj_d"]) * ln1[None, :])
    wqT, f_q = tq(np.asarray(inputs["q_proj_d"]) * ln1[None, :])
    wrkT, f_rk = tq(np.asarray(inputs["rope_k"]) * ln1[None, :])
    wvT, f_v = tq(np.asarray(inputs["v_proj_u"]))
    wrqT, f_rq = tq(np.asarray(inputs["rope_q"]))
    woT, f_o = tq(np.asarray(inputs["o_proj"]))
    wrtT = np.ascontiguousarray(
        (np.asarray(inputs["router_w"], np.float32) * ln2[None, :]).T
        .reshape(HC, P, E).transpose(1, 0, 2))
    rbias = np.asarray(inputs["routing_bias"], np.float32).reshape(1, E)

    wsgT = tb(np.asarray(inputs["sh_gate"]) * ln2[None, :])
    wsuT = tb(np.asarray(inputs["sh_up"]) * ln2[None, :])
    wsdT = tb(np.asarray(inputs["sh_down"]))
    eg_l = [tq(np.asarray(inputs["ex_gate"][e]) * ln2[None, :]) for e in range(E)]
    eu_l = [tq(np.asarray(inputs["ex_up"][e]) * ln2[None, :]) for e in range(E)]
    ed_l = [tq(np.asarray(inputs["ex_down"][e])) for e in range(E)]
    wegT = np.stack([q for q, _ in eg_l])
    weuT = np.stack([q for q, _ in eu_l])
    wedT = np.stack([q for q, _ in ed_l])
    scales = {"eg": [f for _, f in eg_l], "eu": [f for _, f in eu_l],
              "ed": [f for _, f in ed_l],
              "kv": f_kv, "q": f_q, "v": f_v, "rq": f_rq, "rk": f_rk,
              "o": f_o}

    cosT, sinT = _rope_tables()
    cosb = (cosT * C_K).astype(bf)   # pre-scaled: krope lands at C_K*k
    sinb = (sinT * C_K).astype(bf)
    cosq = (cosT * C_Q).astype(bf)   # pre-scaled: qrope lands at C_Q*q
    sinq = (sinT * C_Q).astype(bf)

    xT = np.ascontiguousarray(x.transpose(0, 2, 1)).astype(bf)  # [B, H, T]
    iob_np = np.tile(np.arange(256, dtype=np.float32), (P, 1))
    triS_np = np.tril(np.ones((P, P), np.float32), -1).astype(bf)
    ones2d_np = np.ones((P, P), np.float32).astype(bf)
    f8 = ml_dtypes.float8_e4m3

    in_maps = []
    for c in range(N_CORES):
        b, j = c // 4, c % 4
        qoff = 512 * j
        kk = np.arange(TB).reshape(KC, P, 1)
        qq = qoff + np.arange(TT).reshape(1, 1, TT)
        msk = (kk <= qq).astype(f8)
        in_maps.append({
            "xbT": xT[b],
            "xoT": np.ascontiguousarray(xT[b][:, qoff:qoff + TT]),
            "xon": np.ascontiguousarray(x[b][qoff:qoff + TT, :]),
            "cosb": cosb, "sinb": sinb,
            "cosq": np.ascontiguousarray(cosq[:, qoff:qoff + TT]),
            "sinq": np.ascontiguousarray(sinq[:, qoff:qoff + TT]),
            "msk": msk,
            "wkvT": wkvT, "wqT": wqT, "wvT": wvT, "wrqT": wrqT,
            "wrkT": wrkT, "woT": woT, "wrtT": wrtT, "rbias": rbias,
            "wsgT": wsgT, "wsuT": wsuT, "wsdT": wsdT,
            "wegT": wegT, "weuT": weuT, "wedT": wedT,
            "iob": iob_np, "triS": triS_np, "ones2d": ones2d_np,
        })
    return in_maps, scales


def kernel(**inputs):
    in_maps, scales = make_in_maps(inputs)
    import os
    nc = _get_nc(scales)
    trace = bool(int(os.environ.get("KERNEL_TRACE", "0")))
    res = run_bass_kernel_spmd(nc, in_maps, core_ids=list(range(N_CORES)),
                               trace=trace,
                               trace_cores=[0, 3, 7] if trace else None)
    _NC_CACHE["last_result"] = res

    outp = np.empty((B, T, H), np.float32)
    for c in range(N_CORES):
        b, j = c // 4, c % 4
        outp[b, 512 * j:512 * (j + 1), :] = res.results[c]["out"]
    return outp

